# revision 1
# baseline (speedup 1.0000x reference)
"""Multi-head GAT layer on 8 Trainium2 NeuronCores (Bass/Tile SPMD kernel).

Strategy (edge-parallel, target-sharded):
  - Edges sorted by target, sharded across 8 cores by contiguous target
    ranges (N/8 nodes each): softmax + aggregation are core-local.
  - Phase 1 (replicated on every core): one bf16 PE pass over the node
    features builds an augmented per-node table row
      [ h (128) | s2 (8) | s1 (8) | deg (1) | pad ]  (bf16, 512B rows)
    where h = NF @ W.T + b and s1/s2 are the per-node attention scores
    h . a1 / h . a2 (fused into the same matmul via W.T @ A12).
  - Phase 2: per 128-target block, edge slots (padded to 128-slot tiles,
    sorted by src) are fetched with dma_gather (int16 indices + a static
    per-group base, 4 SWDGE queues round-robin).  Per tile, a one-hot
    matrix maps slots to local target rows; PE matmuls accumulate both
    the weighted message sum and the softmax denominator in PSUM.  The
    softmax division is pulled out of the edge loop (out = acc/denom);
    the skip term (deg * h_i) rides along as a per-target self-slot
    whose edge weight is deg * denom, so it survives the division
    exactly.  ELU finalize in fp32; contiguous output writes.
"""

import numpy as np

N_CORES = 8
_last_results = None  # BassKernelResults of the most recent run (for harnesses)


def _install_ntff_hook():
    """Register the axon NTFF profiling hook if the image lacks antenv.axon_hooks."""
    import sys, types
    try:
        from antenv.axon_hooks import get_axon_ntff_profile_hook  # noqa: F401
        return
    except ImportError:
        pass
    try:
        mod = types.ModuleType("antenv.axon_hooks")
        holder = [None]
        mod.set_axon_ntff_profile_hook = lambda h: holder.__setitem__(0, h)
        mod.get_axon_ntff_profile_hook = lambda: holder[0]
        sys.modules["antenv.axon_hooks"] = mod
        from trn_agent_boot.trn_boot import _ntff_profile_via_ctypes
        mod.set_axon_ntff_profile_hook(
            _ntff_profile_via_ctypes("/opt/axon/libaxon_pjrt.so"))
    except Exception:
        sys.modules.pop("antenv.axon_hooks", None)


def kernel(node_features, edge_index, W, b, a):
    return gat_multicore(
        np.asarray(node_features, dtype=np.float32),
        np.asarray(edge_index, dtype=np.int32),
        np.asarray(W, dtype=np.float32),
        np.asarray(b, dtype=np.float32),
        np.asarray(a, dtype=np.float32),
    )


def gat_multicore(nf, ei, W, b, a, slope=0.2):
    import sys
    if "/opt/trn_rl_repo" not in sys.path:
        sys.path.insert(0, "/opt/trn_rl_repo")
    import ml_dtypes
    import concourse.bacc as bacc
    import concourse.tile as tile
    import concourse.mybir as mybir
    from concourse import library_config
    from concourse.bass import IndirectOffsetOnAxis
    from concourse.bass_utils import run_bass_kernel_spmd
    from contextlib import ExitStack

    fp32 = mybir.dt.float32
    bf16 = mybir.dt.bfloat16
    i32 = mybir.dt.int32
    i16 = mybir.dt.int16
    AF = mybir.ActivationFunctionType
    OP = mybir.AluOpType
    bfnp = ml_dtypes.bfloat16

    N, F_IN = nf.shape
    E = ei.shape[1]
    HF = W.shape[0]               # H * F_OUT
    F_OUT = a.shape[0] // 2
    H = HF // F_OUT
    assert F_IN == 128 and HF == 128, "kernel assumes 128 in/out features"
    assert N % N_CORES == 0
    NPC = N // N_CORES            # targets per core
    NBLK = (NPC + 127) // 128     # 128-target blocks per core
    GRP = 8                       # max tiles per gather group
    ROW = 256                     # bf16 elements per table row (512 B)
    SPAN = 30000                  # max int16 index span per gather group

    # ---------------- host prep: weights ----------------
    WT = np.ascontiguousarray(W.T)                       # [F_IN, HF]
    # A12 column order: [s2 (a2) | s1 (a1)] to match the table row layout
    A12 = np.zeros((HF, 2 * H), dtype=np.float32)
    for hd in range(H):
        A12[hd * F_OUT:(hd + 1) * F_OUT, hd] = a[F_OUT:]        # s2
        A12[hd * F_OUT:(hd + 1) * F_OUT, H + hd] = a[:F_OUT]    # s1
    M12 = (WT @ A12).astype(np.float32)                  # [F_IN, 2H]
    b12 = (b @ A12).astype(np.float32)                   # [2H]
    b_ext = np.concatenate([b, b12]).astype(np.float32)  # [144]
    b_rep = np.broadcast_to(b_ext, (128, HF + 2 * H)).copy()
    NFT = np.ascontiguousarray(nf.T).astype(bfnp)        # [F_IN, N] bf16
    iota_rep = np.broadcast_to(
        np.arange(128, dtype=np.float32), (128, 128)).astype(bfnp).copy()
    ident = np.eye(128, dtype=np.float32).astype(bfnp)

    # ---------------- host prep: graph structure ----------------
    src, tgt = ei[0].astype(np.int64), ei[1].astype(np.int64)
    order = np.argsort(tgt, kind="stable")
    ssrc, stgt = src[order], tgt[order]
    deg_full = np.bincount(tgt, minlength=N).astype(np.float32)
    n_nt = (N + 127) // 128
    deg_pad = np.zeros(n_nt * 128, dtype=np.float32)
    deg_pad[:N] = deg_full
    deg_wrap = deg_pad.reshape(n_nt, 128).T.astype(bfnp).copy()

    blk_bounds = []
    for c in range(N_CORES):
        bounds = [c * NPC + bb * 128 for bb in range(NBLK)] + [(c + 1) * NPC]
        blk_bounds.append(np.searchsorted(stgt, bounds))
    cnt = np.array([[blk_bounds[c][bb + 1] - blk_bounds[c][bb]
                     for bb in range(NBLK)] for c in range(N_CORES)])
    # edge tiles per block (uniform across cores), +1 self tile
    n_edge_tiles = np.maximum(1, (cnt.max(axis=0) + 127) // 128)
    n_tiles_blk = n_edge_tiles + 1
    NT = int(n_tiles_blk.sum())
    t_ofs_blk = np.concatenate([[0], np.cumsum(n_tiles_blk)]).astype(int)

    # Per-core slot arrays; tile t slot p = slot index t*128+p of the block.
    # Last tile of each block is the self-slot tile (gathered by indirect DMA).
    srcs_all = np.zeros((N_CORES, 128, NT), dtype=np.int64)
    rowid_np = np.full((N_CORES, 128, NT), -1.0, dtype=np.float32)
    ownid_np = np.zeros((N_CORES, NBLK, 128), dtype=np.int32)
    for c in range(N_CORES):
        for bb in range(NBLK):
            lo, hi = blk_bounds[c][bb], blk_bounds[c][bb + 1]
            nslot = hi - lo
            base_node = c * NPC + bb * 128
            nrows = min(128, (c + 1) * NPC - base_node)
            t0 = int(t_ofs_blk[bb])
            net = int(n_edge_tiles[bb])
            ne = net * 128
            if nslot > 0:
                o2 = np.argsort(ssrc[lo:hi], kind="stable")
                s_blk = ssrc[lo:hi][o2]
                pad_val = int(s_blk[-1])
                fl_s = np.full(ne, pad_val, dtype=np.int64)
                fl_r = np.full(ne, -1.0, dtype=np.float32)
                fl_s[:nslot] = s_blk
                fl_r[:nslot] = (stgt[lo:hi][o2] - base_node).astype(np.float32)
                srcs_all[c, :, t0:t0 + net] = fl_s.reshape(net, 128).T
                rowid_np[c, :, t0:t0 + net] = fl_r.reshape(net, 128).T
            # else: pad_val filled below from other cores
            own = np.arange(128)
            valid = own < nrows
            ownid_np[c, bb] = np.where(valid, base_node + own, base_node)
            srcs_all[c, :, t0 + net] = 0  # unused (indirect gather path)
            rowid_np[c, :, t0 + net] = np.where(valid, own.astype(np.float32), -1.0)
    # blocks empty on some core but not others: align pad values to the
    # cross-core span by borrowing any non-empty core's pad value
    for bb in range(NBLK):
        t0 = int(t_ofs_blk[bb])
        net = int(n_edge_tiles[bb])
        nonempty = [c for c in range(N_CORES) if cnt[c][bb] > 0]
        if nonempty and len(nonempty) < N_CORES:
            ref = int(srcs_all[nonempty[0], 0, t0])
            for c in range(N_CORES):
                if cnt[c][bb] == 0:
                    srcs_all[c, :, t0:t0 + net] = ref

    # Gather groups over EDGE tiles only: consecutive tiles of one block,
    # <= GRP tiles, cross-core index span <= SPAN.  Base per group is the
    # cross-core min -> static program constant.
    groups = []          # (block, tile_lo, n_tiles, base)
    for bb in range(NBLK):
        net = int(n_edge_tiles[bb])
        t0 = int(t_ofs_blk[bb])
        t = 0
        while t < net:
            best = 1
            for w in range(2, min(GRP, net - t) + 1):
                sl = srcs_all[:, :, t0 + t:t0 + t + w]
                if sl.max() - sl.min() > SPAN:
                    break
                best = w
            sl = srcs_all[:, :, t0 + t:t0 + t + best]
            assert sl.max() - sl.min() <= 32000, "single tile span too large"
            groups.append((bb, t, best, int(sl.min())))
            t += best

    g_cols = [(g[2] * 128) // 16 for g in groups]
    g_col_ofs = np.concatenate([[0], np.cumsum(g_cols)]).astype(int)
    IDXC = int(g_col_ofs[-1])
    idx16_np = np.zeros((N_CORES, 128, IDXC), dtype=np.int16)
    for c in range(N_CORES):
        for gi, (bb, tl, w, base) in enumerate(groups):
            t0 = int(t_ofs_blk[bb]) + tl
            rel = (srcs_all[c, :, t0:t0 + w] - base).astype(np.int16)  # [128, w]
            flat = rel.T.reshape(-1)                 # slot order t*128+p
            wrapped = flat.reshape(-1, 16).T         # [16, w*128/16]
            idx16_np[c, :, g_col_ofs[gi]:g_col_ofs[gi + 1]] = np.tile(wrapped, (8, 1))

    rowid_bf = rowid_np.astype(bfnp)

    # ---------------- build the SPMD program ----------------
    nc = bacc.Bacc("TRN2", target_bir_lowering=False, debug=False,
                   num_devices=N_CORES, num_swdge_queues=4)

    nft_d = nc.dram_tensor("nft", [128, n_nt * 128], bf16, kind="ExternalInput").ap()
    wt_d = nc.dram_tensor("wt", [128, HF], bf16, kind="ExternalInput").ap()
    m12_d = nc.dram_tensor("m12", [128, 2 * H], bf16, kind="ExternalInput").ap()
    brep_d = nc.dram_tensor("brep", [128, HF + 2 * H], fp32, kind="ExternalInput").ap()
    iota_d = nc.dram_tensor("iota", [128, 128], bf16, kind="ExternalInput").ap()
    ident_d = nc.dram_tensor("ident", [128, 128], bf16, kind="ExternalInput").ap()
    degw_d = nc.dram_tensor("degw", [128, n_nt], bf16, kind="ExternalInput").ap()
    idx16_d = nc.dram_tensor("idx16", [128, IDXC], i16, kind="ExternalInput").ap()
    rowid_d = nc.dram_tensor("rowid", [128, NT], bf16, kind="ExternalInput").ap()
    ownid_d = nc.dram_tensor("ownid", [NBLK, 128], i32, kind="ExternalInput").ap()

    h_tab = nc.dram_tensor("h_tab", [N, ROW], bf16).ap()
    out_d = nc.dram_tensor("out", [NPC, HF], fp32, kind="ExternalOutput").ap()

    CW = HF + H       # 136: [Msg | ex] combo width
    SW = HF + 2 * H   # 144: phase-1 psum width
    MAXT = int(n_tiles_blk.max())

    with tile.TileContext(nc) as tc:
        with ExitStack() as ctx:
            cpool = ctx.enter_context(tc.tile_pool(name="consts", bufs=1))
            p1 = ctx.enter_context(tc.tile_pool(name="p1", bufs=3))
            p1ps = ctx.enter_context(tc.tile_pool(name="p1ps", bufs=1, space="PSUM"))
            gp = ctx.enter_context(tc.tile_pool(name="gather", bufs=2))
            mp = ctx.enter_context(tc.tile_pool(name="meta", bufs=3))
            ps_acc = ctx.enter_context(tc.tile_pool(name="ps_acc", bufs=2, space="PSUM"))
            ps_z = ctx.enter_context(tc.tile_pool(name="ps_z", bufs=2, space="PSUM"))
            ps_t = ctx.enter_context(tc.tile_pool(name="ps_t", bufs=1, space="PSUM"))
            fin = ctx.enter_context(tc.tile_pool(name="fin", bufs=2))

            nc.gpsimd.load_library(library_config.mlp)

            wt_sb = cpool.tile([128, HF], bf16)
            nc.sync.dma_start(wt_sb[:], wt_d[:])
            m12_sb = cpool.tile([128, 2 * H], bf16)
            nc.sync.dma_start(m12_sb[:], m12_d[:])
            brep_sb = cpool.tile([128, SW], fp32)
            nc.sync.dma_start(brep_sb[:], brep_d[:])
            iota_sb = cpool.tile([128, 128], bf16)
            nc.sync.dma_start(iota_sb[:], iota_d[:])
            ident_sb = cpool.tile([128, 128], bf16)
            nc.sync.dma_start(ident_sb[:], ident_d[:])
            idx_sb = cpool.tile([128, IDXC], i16)
            nc.sync.dma_start(idx_sb[:], idx16_d[:])

            # ---------- phase 1: augmented h table (replicated) ----------
            b_is_zero = not np.any(b_ext)
            CH = 512
            for j0 in range(0, N, CH):
                w = min(CH, N - j0)
                nfc = p1.tile([128, CH], bf16, tag="nfc")
                nc.sync.dma_start(nfc[:, :w], nft_d[:, j0:j0 + w])
                ncols = (w + 127) // 128
                degc = p1.tile([128, CH // 128], bf16, tag="degc")
                nc.sync.dma_start(degc[:, :ncols],
                                  degw_d[:, j0 // 128:j0 // 128 + ncols])
                for k0 in range(0, w, 256):
                    # two node-tiles per hrow buffer / table write
                    kw2 = min(256, w - k0)
                    hrow = p1.tile([128, 2, ROW], bf16, tag="hrow")
                    nk = (kw2 + 127) // 128
                    ps = p1ps.tile([128, 2, SW], fp32, space="PSUM", tag="p1ps")
                    for k in range(nk):
                        kk = k0 + k * 128
                        kw = min(128, w - kk)
                        nc.tensor.matmul(ps[:kw, k, 0:HF],
                                         lhsT=nfc[:, kk:kk + kw],
                                         rhs=wt_sb[:], start=True, stop=True)
                        nc.tensor.matmul(ps[:kw, k, HF:SW],
                                         lhsT=nfc[:, kk:kk + kw],
                                         rhs=m12_sb[:], start=True, stop=True)
                    if b_is_zero:
                        nc.vector.tensor_copy(hrow[:, :nk, 0:SW], ps[:, :nk, :])
                    else:
                        nc.vector.tensor_tensor(
                            out=hrow[:, :nk, 0:SW], in0=ps[:, :nk, :],
                            in1=brep_sb[:].unsqueeze(1).broadcast_to([128, nk, SW]),
                            op=OP.add)
                    nc.vector.tensor_copy(
                        hrow[:, :nk, SW:SW + 1],
                        degc[:, k0 // 128:k0 // 128 + nk].unsqueeze(2))
                    n0 = j0 + k0
                    if kw2 == nk * 128:
                        nc.scalar.dma_start(
                            h_tab[n0:n0 + kw2, :].rearrange(
                                "(k p) r -> p k r", k=nk),
                            hrow[:, :nk, :])
                    else:
                        nc.scalar.dma_start(h_tab[n0:n0 + kw2, :],
                                            hrow[:kw2, 0, :])

            # ---------- phase 2: edge processing ----------
            # Per block: main = gathers + scores + weighted-agg matmuls into
            # PSUM; tail = softmax division + skip + ELU + output write.
            # Tails are emitted one block late so their PSUM waits overlap
            # with the next block's compute.
            blk_state = {}

            def emit_main(bb, qn0):
                ntb = int(n_tiles_blk[bb])
                net = ntb - 1
                t0 = int(t_ofs_blk[bb])
                Tself = net
                qn = qn0

                acc = ps_acc.tile([128, CW], fp32, space="PSUM", tag="acc")
                accd = ps_acc.tile([128, H], fp32, space="PSUM", tag="accd")
                G = gp.tile([128, MAXT, ROW], bf16, tag="G")
                msg = gp.tile([128, MAXT, HF], bf16, tag="msg")
                ext = gp.tile([128, MAXT, H], bf16, tag="ext")
                oh = gp.tile([128, MAXT, 128], bf16, tag="oh")
                ridt = mp.tile([128, MAXT], bf16, tag="ridt")
                nc.sync.dma_start(ridt[:, :ntb], rowid_d[:, t0:t0 + ntb])

                # self tile gather (absolute int32 ids, per-core data)
                ownt = mp.tile([128, 1], i32, tag="ownt")
                nc.sync.dma_start(ownt[:, :], ownid_d[bb, :].unsqueeze(1))
                nc.gpsimd.indirect_dma_start(
                    out=G[:, Tself, :], out_offset=None, in_=h_tab[:, :],
                    in_offset=IndirectOffsetOnAxis(ap=ownt[:, 0:1], axis=0))

                for gi, (gbb, tl, wdt, base) in enumerate(groups):
                    if gbb != bb:
                        continue
                    nc.gpsimd.dma_gather(
                        out_ap=G[:, tl:tl + wdt, :],
                        in_ap=h_tab[base:, :],
                        idxs_ap=idx_sb[:, g_col_ofs[gi]:g_col_ofs[gi + 1]],
                        num_idxs=wdt * 128, num_idxs_reg=wdt * 128,
                        elem_size=ROW, queue_num=qn % 4)
                    qn += 1

                # one-hot (bf16) for the edge tiles, batched
                nc.vector.tensor_tensor(
                    out=oh[:, :net, :],
                    in0=ridt[:, :net].unsqueeze(2).broadcast_to([128, net, 128]),
                    in1=iota_sb[:].unsqueeze(1).broadcast_to([128, net, 128]),
                    op=OP.is_equal)

                # s1 of the block's targets from the self tile's rows
                s1_blk = G[:, Tself, SW - H:SW]          # [128, H] bf16

                # per-tile: transpose(one-hot) -> s1e matmul
                zps = ps_z.tile([128, MAXT, H], fp32, space="PSUM", tag="zps")
                TGRP = 8
                for q0 in range(0, net, TGRP):
                    qw = min(TGRP, net - q0)
                    ohT_ps = ps_t.tile([128, TGRP, 128], bf16, space="PSUM",
                                       tag="ohT_ps")
                    for t in range(q0, q0 + qw):
                        nc.tensor.transpose(ohT_ps[:, t - q0, :], in_=oh[:, t, :],
                                            identity=ident_sb[:])
                    ohT_sb = gp.tile([128, TGRP, 128], bf16, tag="ohT_sb")
                    nc.scalar.copy(ohT_sb[:, :qw, :], ohT_ps[:, :qw, :])
                    for t in range(q0, q0 + qw):
                        nc.tensor.matmul(zps[:, t, :], lhsT=ohT_sb[:, t - q0, :],
                                         rhs=s1_blk, start=True, stop=True)

                # ex = exp(leakyrelu(s1e + s2)); z = zps + G.s2 on DVE
                z_sb = mp.tile([128, MAXT, H], fp32, tag="z_sb")
                nc.vector.tensor_tensor(out=z_sb[:, :net, :], in0=zps[:, :net, :],
                                        in1=G[:, :net, HF:HF + H], op=OP.add)
                nc.vector.scalar_tensor_tensor(
                    out=ext[:, :net, :], in0=z_sb[:, :net, :], scalar=slope,
                    in1=z_sb[:, :net, :], op0=OP.mult, op1=OP.max)
                nc.scalar.activation(ext[:, :net, :], ext[:, :net, :], AF.Exp)
                # Msg = ex * h_src for the edge tiles
                nc.vector.tensor_tensor(
                    out=msg[:, 0:net, :], in0=G[:, 0:net, 0:HF],
                    in1=ext[:, 0:net, :].unsqueeze(3).broadcast_to(
                        [128, net, H, F_OUT]),
                    op=OP.mult)

                for t in range(net):
                    nc.tensor.matmul(acc[:, 0:HF], lhsT=oh[:, t, :],
                                     rhs=msg[:, t, :],
                                     start=(t == 0), stop=(t == net - 1))
                    nc.tensor.matmul(accd[:, :], lhsT=oh[:, t, :],
                                     rhs=ext[:, t, :],
                                     start=(t == 0), stop=(t == net - 1))

                blk_state[bb] = (acc, accd, G, Tself)
                return qn

            def emit_tail(bb):
                ntb = int(n_tiles_blk[bb])
                base_row = bb * 128
                nrows = min(128, NPC - base_row)
                acc, accd, G, Tself = blk_state.pop(bb)

                rec = fin.tile([128, H], fp32, tag="rec")
                nc.vector.tensor_scalar_add(out=rec[:, :], in0=accd[:, :],
                                            scalar1=1e-30)
                nc.vector.reciprocal(rec[:, :], rec[:, :])
                nrm = fin.tile([128, HF], fp32, tag="nrm")
                nc.vector.tensor_tensor(
                    out=nrm[:, :], in0=acc[:, 0:HF],
                    in1=rec[:].unsqueeze(2).broadcast_to([128, H, F_OUT]),
                    op=OP.mult)
                # += deg * h_own (fp32, exact skip term)
                deg_col = fin.tile([128, 1], fp32, tag="deg_col")
                nc.vector.tensor_copy(deg_col[:], G[:, Tself, SW:SW + 1])
                nc.vector.scalar_tensor_tensor(
                    out=nrm[:, :], in0=G[:, Tself, 0:HF], scalar=deg_col[:, 0:1],
                    in1=nrm[:, :], op0=OP.mult, op1=OP.add)
                # ELU = max(x,0) + exp(min(x,0)) - 1
                neg = fin.tile([128, HF], fp32, tag="neg")
                nc.vector.tensor_scalar_min(out=neg[:, :], in0=nrm[:, :], scalar1=0.0)
                nc.scalar.activation(neg[:, :], neg[:, :], AF.Exp)
                pos = fin.tile([128, HF], fp32, tag="pos")
                nc.vector.tensor_scalar_max(out=pos[:, :], in0=nrm[:, :], scalar1=0.0)
                res = fin.tile([128, HF], fp32, tag="res")
                nc.vector.scalar_tensor_tensor(
                    out=res[:, :], in0=neg[:, :], scalar=-1.0, in1=pos[:, :],
                    op0=OP.add, op1=OP.add)
                nc.scalar.dma_start(out_d[base_row:base_row + nrows, :],
                                    res[:nrows, :])

            qn = 0
            for bb in range(NBLK):
                qn = emit_main(bb, qn)
                if bb > 0:
                    emit_tail(bb - 1)
            emit_tail(NBLK - 1)

    nc.compile()

    in_maps = []
    for c in range(N_CORES):
        in_maps.append({
            "nft": _pad_cols(NFT, n_nt * 128), "wt": WT.astype(bfnp),
            "m12": M12.astype(bfnp), "brep": b_rep, "iota": iota_rep,
            "ident": ident, "degw": deg_wrap,
            "idx16": idx16_np[c], "rowid": rowid_bf[c], "ownid": ownid_np[c],
        })
    import os
    trace = bool(os.environ.get("GAT_TRACE"))
    if trace:
        _install_ntff_hook()
    res = run_bass_kernel_spmd(nc, in_maps, list(range(N_CORES)), trace=trace)
    global _last_results
    _last_results = res
    out = np.concatenate([res.results[c]["out"] for c in range(N_CORES)], axis=0)
    return out


def _pad_cols(arr, cols):
    if arr.shape[1] == cols:
        return arr
    out = np.zeros((arr.shape[0], cols), dtype=arr.dtype)
    out[:, :arr.shape[1]] = arr
    return out



# revision 10
# speedup vs baseline: 1.2416x; 1.2416x over previous
"""Multi-head GAT layer on 8 Trainium2 NeuronCores (Bass/Tile SPMD kernel).

Strategy (edge-parallel, target-sharded):
  - Edges sorted by target, sharded across 8 cores by contiguous target
    ranges (N/8 nodes each): softmax + aggregation are core-local.
  - Phase 1a (replicated on every core): one bf16 PE pass over the node
    features builds an augmented per-node table row
      [ h (128) | s2 (8) | s1 (8) | deg (1) | pad ]  (bf16, 512B rows)
    where h = NF @ W.T + b and s1/s2 are the per-node attention scores
    h . a1 / h . a2 (fused into the same matmul via W.T @ A12).
  - Phase 1b (per-core data, same program): the core's own 6250 target
    rows are recomputed into a resident SBUF table (fp32) so phase 2
    needs no self-row gather at all.
  - Phase 2: per 128-target block, edge slots (padded to 128-slot tiles,
    sorted by src) are fetched with dma_gather (int16 indices + a static
    per-group base, 4 SWDGE queues round-robin, 64KB descriptor rings so
    the Q7 never stalls on ring space).  Per tile, a one-hot matrix maps
    slots to local target rows; a single PE matmul per tile accumulates
    the weighted message sum and the softmax denominator together in
    PSUM ([Msg | ex] combo rhs).  The softmax division is pulled out of
    the edge loop (out = acc/denom); the skip term deg * h_i is added
    after the division from the SBUF-resident own-row table.  ELU
    finalize in fp32; contiguous output writes.
"""

import numpy as np

N_CORES = 8
_last_results = None  # BassKernelResults of the most recent run (for harnesses)


def _install_ntff_hook():
    """Register the axon NTFF profiling hook if the image lacks antenv.axon_hooks."""
    import sys, types
    try:
        from antenv.axon_hooks import get_axon_ntff_profile_hook  # noqa: F401
        return
    except ImportError:
        pass
    try:
        mod = types.ModuleType("antenv.axon_hooks")
        holder = [None]
        mod.set_axon_ntff_profile_hook = lambda h: holder.__setitem__(0, h)
        mod.get_axon_ntff_profile_hook = lambda: holder[0]
        sys.modules["antenv.axon_hooks"] = mod
        from trn_agent_boot.trn_boot import _ntff_profile_via_ctypes
        mod.set_axon_ntff_profile_hook(
            _ntff_profile_via_ctypes("/opt/axon/libaxon_pjrt.so"))
    except Exception:
        sys.modules.pop("antenv.axon_hooks", None)


def kernel(node_features, edge_index, W, b, a):
    return gat_multicore(
        np.asarray(node_features, dtype=np.float32),
        np.asarray(edge_index, dtype=np.int32),
        np.asarray(W, dtype=np.float32),
        np.asarray(b, dtype=np.float32),
        np.asarray(a, dtype=np.float32),
    )


def gat_multicore(nf, ei, W, b, a, slope=0.2):
    import sys
    if "/opt/trn_rl_repo" not in sys.path:
        sys.path.insert(0, "/opt/trn_rl_repo")
    import ml_dtypes
    import concourse.bacc as bacc
    import concourse.tile as tile
    import concourse.mybir as mybir
    from concourse import library_config
    from concourse.bass_utils import run_bass_kernel_spmd
    from contextlib import ExitStack

    fp32 = mybir.dt.float32
    bf16 = mybir.dt.bfloat16
    i16 = mybir.dt.int16
    AF = mybir.ActivationFunctionType
    OP = mybir.AluOpType
    bfnp = ml_dtypes.bfloat16

    N, F_IN = nf.shape
    E = ei.shape[1]
    HF = W.shape[0]               # H * F_OUT
    F_OUT = a.shape[0] // 2
    H = HF // F_OUT
    assert F_IN == 128 and HF == 128, "kernel assumes 128 in/out features"
    assert N % N_CORES == 0
    NPC = N // N_CORES            # targets per core
    NBLK = (NPC + 127) // 128     # 128-target blocks per core
    GRP = 8                       # max tiles per gather group
    ROW = 256                     # bf16 elements per table row (512 B)
    SPAN = 30000                  # max int16 index span per gather group

    # ---------------- host prep: weights ----------------
    WT = np.ascontiguousarray(W.T)                       # [F_IN, HF]
    # A12 column order: [s2 (a2) | s1 (a1)] to match the table row layout
    A12 = np.zeros((HF, 2 * H), dtype=np.float32)
    for hd in range(H):
        A12[hd * F_OUT:(hd + 1) * F_OUT, hd] = a[F_OUT:]        # s2
        A12[hd * F_OUT:(hd + 1) * F_OUT, H + hd] = a[:F_OUT]    # s1
    M12 = (WT @ A12).astype(np.float32)                  # [F_IN, 2H]
    b12 = (b @ A12).astype(np.float32)                   # [2H]
    b_ext = np.concatenate([b, b12]).astype(np.float32)  # [144]
    b_rep = np.broadcast_to(b_ext, (128, HF + 2 * H)).copy()
    NFT = np.ascontiguousarray(nf.T).astype(bfnp)        # [F_IN, N] bf16
    iota_rep = np.broadcast_to(
        np.arange(128, dtype=np.float32), (128, 128)).astype(bfnp).copy()
    ident = np.eye(128, dtype=np.float32).astype(bfnp)

    # ---------------- host prep: graph structure ----------------
    src, tgt = ei[0].astype(np.int64), ei[1].astype(np.int64)
    order = np.argsort(tgt, kind="stable")
    ssrc, stgt = src[order], tgt[order]
    deg_full = np.bincount(tgt, minlength=N).astype(np.float32)
    n_nt = (N + 127) // 128
    NPAD = n_nt * 128             # h_tab rows incl. zero padding
    deg_pad = np.zeros(NPAD, dtype=np.float32)
    deg_pad[:N] = deg_full
    deg_wrap = deg_pad.reshape(n_nt, 128).T.astype(bfnp).copy()

    blk_bounds = []
    for c in range(N_CORES):
        bounds = [c * NPC + bb * 128 for bb in range(NBLK)] + [(c + 1) * NPC]
        blk_bounds.append(np.searchsorted(stgt, bounds))
    cnt = np.array([[blk_bounds[c][bb + 1] - blk_bounds[c][bb]
                     for bb in range(NBLK)] for c in range(N_CORES)])
    # edge tiles per block (uniform across cores)
    n_tiles_blk = np.maximum(1, (cnt.max(axis=0) + 127) // 128)
    NT = int(n_tiles_blk.sum())
    t_ofs_blk = np.concatenate([[0], np.cumsum(n_tiles_blk)]).astype(int)

    # Per-core slot arrays; tile t slot p = slot index t*128+p of the block.
    srcs_all = np.zeros((N_CORES, 128, NT), dtype=np.int64)
    rowid_np = np.full((N_CORES, 128, NT), -1.0, dtype=np.float32)
    for c in range(N_CORES):
        for bb in range(NBLK):
            lo, hi = blk_bounds[c][bb], blk_bounds[c][bb + 1]
            nslot = hi - lo
            base_node = c * NPC + bb * 128
            t0 = int(t_ofs_blk[bb])
            net = int(n_tiles_blk[bb])
            ne = net * 128
            if nslot > 0:
                o2 = np.argsort(ssrc[lo:hi], kind="stable")
                s_blk = ssrc[lo:hi][o2]
                pad_val = int(s_blk[-1])
                fl_s = np.full(ne, pad_val, dtype=np.int64)
                fl_r = np.full(ne, -1.0, dtype=np.float32)
                fl_s[:nslot] = s_blk
                fl_r[:nslot] = (stgt[lo:hi][o2] - base_node).astype(np.float32)
                srcs_all[c, :, t0:t0 + net] = fl_s.reshape(net, 128).T
                rowid_np[c, :, t0:t0 + net] = fl_r.reshape(net, 128).T
            # else: pad filled below from other cores
    # blocks empty on some core but not others: align pad values to the
    # cross-core span by borrowing any non-empty core's pad value
    for bb in range(NBLK):
        t0 = int(t_ofs_blk[bb])
        net = int(n_tiles_blk[bb])
        nonempty = [c for c in range(N_CORES) if cnt[c][bb] > 0]
        if nonempty and len(nonempty) < N_CORES:
            ref = int(srcs_all[nonempty[0], 0, t0])
            for c in range(N_CORES):
                if cnt[c][bb] == 0:
                    srcs_all[c, :, t0:t0 + net] = ref

    # Gather groups: consecutive tiles of one block, <= GRP tiles,
    # cross-core index span <= SPAN.  Base per group is the cross-core
    # min -> static program constant.
    groups = []          # (block, tile_lo, n_tiles, base)
    for bb in range(NBLK):
        net = int(n_tiles_blk[bb])
        t0 = int(t_ofs_blk[bb])
        t = 0
        while t < net:
            best = 1
            for w in range(2, min(GRP, net - t) + 1):
                sl = srcs_all[:, :, t0 + t:t0 + t + w]
                if sl.max() - sl.min() > SPAN:
                    break
                best = w
            sl = srcs_all[:, :, t0 + t:t0 + t + best]
            assert sl.max() - sl.min() <= 32000, "single tile span too large"
            groups.append((bb, t, best, int(sl.min())))
            t += best
    groups_by_block = [[] for _ in range(NBLK)]
    for gi, g in enumerate(groups):
        groups_by_block[g[0]].append((gi,) + g[1:])

    g_cols = [(g[2] * 128) // 16 for g in groups]
    g_col_ofs = np.concatenate([[0], np.cumsum(g_cols)]).astype(int)
    IDXC = int(g_col_ofs[-1])
    idx16_np = np.zeros((N_CORES, 128, IDXC), dtype=np.int16)
    for c in range(N_CORES):
        for gi, (bb, tl, w, base) in enumerate(groups):
            t0 = int(t_ofs_blk[bb]) + tl
            rel = (srcs_all[c, :, t0:t0 + w] - base).astype(np.int16)  # [128, w]
            flat = rel.T.reshape(-1)                 # slot order t*128+p
            wrapped = flat.reshape(-1, 16).T         # [16, w*128/16]
            idx16_np[c, :, g_col_ofs[gi]:g_col_ofs[gi + 1]] = np.tile(wrapped, (8, 1))

    rowid_bf = rowid_np.astype(bfnp)

    # Per-core own-node inputs for phase 1b (the core's target rows)
    OWNW = NBLK * 128
    nft_own_np = np.zeros((N_CORES, 128, OWNW), dtype=bfnp)
    deg_own_np = np.zeros((N_CORES, 128, NBLK), dtype=bfnp)
    for c in range(N_CORES):
        nft_own_np[c, :, :NPC] = NFT[:, c * NPC:(c + 1) * NPC]
        dcol = np.zeros(OWNW, dtype=np.float32)
        dcol[:NPC] = deg_full[c * NPC:(c + 1) * NPC]
        deg_own_np[c] = dcol.reshape(NBLK, 128).T.astype(bfnp)

    # ---------------- build the SPMD program ----------------
    # 64 KiB SWDGE scratch -> 4096-descriptor rings per queue: a full-block
    # 1024-idx gather no longer fills the ring, so the Q7 never stalls in
    # await_space waiting for the previous gather on its queue to drain.
    nc = bacc.Bacc("TRN2", target_bir_lowering=False, debug=False,
                   num_devices=N_CORES, num_swdge_queues=4,
                   dynamic_dma_scratch_size=65536)

    nft_d = nc.dram_tensor("nft", [128, NPAD], bf16, kind="ExternalInput").ap()
    wt_d = nc.dram_tensor("wt", [128, HF], bf16, kind="ExternalInput").ap()
    m12_d = nc.dram_tensor("m12", [128, 2 * H], bf16, kind="ExternalInput").ap()
    brep_d = nc.dram_tensor("brep", [128, HF + 2 * H], fp32, kind="ExternalInput").ap()
    iota_d = nc.dram_tensor("iota", [128, 128], bf16, kind="ExternalInput").ap()
    ident_d = nc.dram_tensor("ident", [128, 128], bf16, kind="ExternalInput").ap()
    degw_d = nc.dram_tensor("degw", [128, n_nt], bf16, kind="ExternalInput").ap()
    idx16_d = nc.dram_tensor("idx16", [128, IDXC], i16, kind="ExternalInput").ap()
    rowid_d = nc.dram_tensor("rowid", [128, NT], bf16, kind="ExternalInput").ap()
    nfto_d = nc.dram_tensor("nft_own", [128, OWNW], bf16, kind="ExternalInput").ap()
    dego_d = nc.dram_tensor("deg_own", [128, NBLK], bf16, kind="ExternalInput").ap()

    h_tab = nc.dram_tensor("h_tab", [NPAD, ROW], bf16).ap()
    out_d = nc.dram_tensor("out", [NPC, HF], fp32, kind="ExternalOutput").ap()

    CW = HF + H       # 136: [Msg | ex] combo width
    SW = HF + 2 * H   # 144: phase-1 psum width
    OSW = SW + 1      # own-row width incl. deg
    MAXT = int(n_tiles_blk.max())

    with tile.TileContext(nc) as tc:
        with ExitStack() as ctx:
            cpool = ctx.enter_context(tc.tile_pool(name="consts", bufs=1))
            p1 = ctx.enter_context(tc.tile_pool(name="p1", bufs=3))
            p1ps = ctx.enter_context(tc.tile_pool(name="p1ps", bufs=2, space="PSUM"))
            gp = ctx.enter_context(tc.tile_pool(name="gather", bufs=3))
            mp = ctx.enter_context(tc.tile_pool(name="meta", bufs=3))
            ps_acc = ctx.enter_context(tc.tile_pool(name="ps_acc", bufs=2, space="PSUM"))
            ps_z = ctx.enter_context(tc.tile_pool(name="ps_z", bufs=2, space="PSUM"))
            ps_t = ctx.enter_context(tc.tile_pool(name="ps_t", bufs=1, space="PSUM"))
            fin = ctx.enter_context(tc.tile_pool(name="fin", bufs=2))

            nc.gpsimd.load_library(library_config.mlp)

            wt_sb = cpool.tile([128, HF], bf16)
            nc.sync.dma_start(wt_sb[:], wt_d[:])
            m12_sb = cpool.tile([128, 2 * H], bf16)
            nc.sync.dma_start(m12_sb[:], m12_d[:])
            brep_sb = cpool.tile([128, SW], fp32)
            nc.sync.dma_start(brep_sb[:], brep_d[:])
            iota_sb = cpool.tile([128, 128], bf16)
            nc.sync.dma_start(iota_sb[:], iota_d[:])
            ident_sb = cpool.tile([128, 128], bf16)
            nc.sync.dma_start(ident_sb[:], ident_d[:])
            idx_sb = cpool.tile([128, IDXC], i16)
            nc.sync.dma_start(idx_sb[:], idx16_d[:])
            rid_all = cpool.tile([128, NT], bf16)
            nc.sync.dma_start(rid_all[:], rowid_d[:])
            dego_sb = cpool.tile([128, NBLK], bf16)
            nc.sync.dma_start(dego_sb[:], dego_d[:])
            # SBUF-resident own-row table [h | s2 | s1 | deg] fp32
            own_sb = cpool.tile([128, NBLK, OSW], fp32)

            b_is_zero = not np.any(b_ext)

            # ---------- phase 1b: own rows -> resident SBUF table ----------
            for ob0 in range(0, NBLK, 2):
                nk = min(2, NBLK - ob0)
                nfo = p1.tile([128, 256], bf16, tag="nfo")
                nc.sync.dma_start(nfo[:, :nk * 128],
                                  nfto_d[:, ob0 * 128:ob0 * 128 + nk * 128])
                ps = p1ps.tile([128, 2, SW], fp32, space="PSUM", tag="p1ps")
                for k in range(nk):
                    nc.tensor.matmul(ps[:, k, 0:HF],
                                     lhsT=nfo[:, k * 128:(k + 1) * 128],
                                     rhs=wt_sb[:], start=True, stop=True)
                    nc.tensor.matmul(ps[:, k, HF:SW],
                                     lhsT=nfo[:, k * 128:(k + 1) * 128],
                                     rhs=m12_sb[:], start=True, stop=True)
                if b_is_zero:
                    nc.vector.tensor_copy(own_sb[:, ob0:ob0 + nk, 0:SW],
                                          ps[:, :nk, :])
                else:
                    nc.vector.tensor_tensor(
                        out=own_sb[:, ob0:ob0 + nk, 0:SW], in0=ps[:, :nk, :],
                        in1=brep_sb[:].unsqueeze(1).broadcast_to([128, nk, SW]),
                        op=OP.add)
                nc.vector.tensor_copy(own_sb[:, ob0:ob0 + nk, SW:SW + 1],
                                      dego_sb[:, ob0:ob0 + nk].unsqueeze(2))

            # ---------- phase 1a: full h table (replicated) ----------
            # Loops over NPAD (zero-padded nodes) so padded rows are finite.
            CH = 512
            for j0 in range(0, NPAD, CH):
                w = min(CH, NPAD - j0)
                nfc = p1.tile([128, CH], bf16, tag="nfc")
                nc.sync.dma_start(nfc[:, :w], nft_d[:, j0:j0 + w])
                ncols = (w + 127) // 128
                degc = p1.tile([128, CH // 128], bf16, tag="degc")
                nc.sync.dma_start(degc[:, :ncols],
                                  degw_d[:, j0 // 128:j0 // 128 + ncols])
                for k0 in range(0, w, 256):
                    # two node-tiles per hrow buffer / table write
                    kw2 = min(256, w - k0)
                    hrow = p1.tile([128, 2, ROW], bf16, tag="hrow")
                    nk = (kw2 + 127) // 128
                    ps = p1ps.tile([128, 2, SW], fp32, space="PSUM", tag="p1ps")
                    for k in range(nk):
                        kk = k0 + k * 128
                        kw = min(128, w - kk)
                        nc.tensor.matmul(ps[:kw, k, 0:HF],
                                         lhsT=nfc[:, kk:kk + kw],
                                         rhs=wt_sb[:], start=True, stop=True)
                        nc.tensor.matmul(ps[:kw, k, HF:SW],
                                         lhsT=nfc[:, kk:kk + kw],
                                         rhs=m12_sb[:], start=True, stop=True)
                    if b_is_zero:
                        nc.vector.tensor_copy(hrow[:, :nk, 0:SW], ps[:, :nk, :])
                    else:
                        nc.vector.tensor_tensor(
                            out=hrow[:, :nk, 0:SW], in0=ps[:, :nk, :],
                            in1=brep_sb[:].unsqueeze(1).broadcast_to([128, nk, SW]),
                            op=OP.add)
                    nc.vector.tensor_copy(
                        hrow[:, :nk, SW:SW + 1],
                        degc[:, k0 // 128:k0 // 128 + nk].unsqueeze(2))
                    n0 = j0 + k0
                    nc.scalar.dma_start(
                        h_tab[n0:n0 + kw2, :].rearrange(
                            "(k p) r -> p k r", k=nk),
                        hrow[:, :nk, :])

            # ---------- phase 2: edge processing ----------
            # Per block: main = gathers + scores + weighted-agg matmuls into
            # PSUM; tail = softmax division + skip + ELU + output write.
            # Tails are emitted one block late so their PSUM waits overlap
            # with the next block's compute.
            blk_state = {}

            def emit_main(bb, qn0):
                net = int(n_tiles_blk[bb])
                t0 = int(t_ofs_blk[bb])
                qn = qn0

                acc = ps_acc.tile([128, CW], fp32, space="PSUM", tag="acc")
                G = gp.tile([128, MAXT, ROW], bf16, tag="G")
                me = gp.tile([128, MAXT, CW], bf16, tag="me")
                oh = gp.tile([128, MAXT, 128], bf16, tag="oh")

                for gi, tl, wdt, base in groups_by_block[bb]:
                    nc.gpsimd.dma_gather(
                        out_ap=G[:, tl:tl + wdt, :],
                        in_ap=h_tab[base:, :],
                        idxs_ap=idx_sb[:, g_col_ofs[gi]:g_col_ofs[gi + 1]],
                        num_idxs=wdt * 128, num_idxs_reg=wdt * 128,
                        elem_size=ROW, queue_num=qn % 4)
                    qn += 1

                # one-hot (bf16) for the edge tiles, batched
                nc.vector.tensor_tensor(
                    out=oh[:, :net, :],
                    in0=rid_all[:, t0:t0 + net].unsqueeze(2).broadcast_to(
                        [128, net, 128]),
                    in1=iota_sb[:].unsqueeze(1).broadcast_to([128, net, 128]),
                    op=OP.is_equal)

                # s1 of the block's targets (bf16 for the PE matmul rhs)
                s1bf = mp.tile([128, H], bf16, tag="s1bf")
                nc.vector.tensor_copy(s1bf[:], own_sb[:, bb, SW - H:SW])

                # per-tile: transpose(one-hot) -> s1e matmul
                zps = ps_z.tile([128, MAXT, H], fp32, space="PSUM", tag="zps")
                TGRP = 8
                for q0 in range(0, net, TGRP):
                    qw = min(TGRP, net - q0)
                    ohT_ps = ps_t.tile([128, TGRP, 128], bf16, space="PSUM",
                                       tag="ohT_ps")
                    for t in range(q0, q0 + qw):
                        nc.tensor.transpose(ohT_ps[:, t - q0, :], in_=oh[:, t, :],
                                            identity=ident_sb[:])
                    ohT_sb = gp.tile([128, TGRP, 128], bf16, tag="ohT_sb")
                    nc.scalar.copy(ohT_sb[:, :qw, :], ohT_ps[:, :qw, :])
                    for t in range(q0, q0 + qw):
                        nc.tensor.matmul(zps[:, t, :], lhsT=ohT_sb[:, t - q0, :],
                                         rhs=s1bf[:], start=True, stop=True)

                # ex = exp(leakyrelu(s1e + s2)); z = zps + G.s2 on DVE
                z_sb = mp.tile([128, MAXT, H], fp32, tag="z_sb")
                nc.vector.tensor_tensor(out=z_sb[:, :net, :], in0=zps[:, :net, :],
                                        in1=G[:, :net, HF:HF + H], op=OP.add)
                ext = mp.tile([128, MAXT, H], bf16, tag="ext")
                nc.vector.scalar_tensor_tensor(
                    out=ext[:, :net, :], in0=z_sb[:, :net, :], scalar=slope,
                    in1=z_sb[:, :net, :], op0=OP.mult, op1=OP.max)
                nc.scalar.activation(ext[:, :net, :], ext[:, :net, :], AF.Exp)
                # combo rhs tile: [Msg | ex] so acc+denom take one matmul/tile
                nc.vector.tensor_copy(me[:, :net, HF:CW], ext[:, :net, :])
                nc.vector.tensor_tensor(
                    out=me[:, 0:net, 0:HF], in0=G[:, 0:net, 0:HF],
                    in1=ext[:, 0:net, :].unsqueeze(3).broadcast_to(
                        [128, net, H, F_OUT]),
                    op=OP.mult)

                for t in range(net):
                    nc.tensor.matmul(acc[:, :], lhsT=oh[:, t, :],
                                     rhs=me[:, t, :],
                                     start=(t == 0), stop=(t == net - 1))

                blk_state[bb] = acc
                return qn

            def emit_tail(bb):
                base_row = bb * 128
                nrows = min(128, NPC - base_row)
                acc = blk_state.pop(bb)

                rec = fin.tile([128, H], fp32, tag="rec")
                nc.vector.tensor_scalar_add(out=rec[:, :], in0=acc[:, HF:CW],
                                            scalar1=1e-30)
                nc.vector.reciprocal(rec[:, :], rec[:, :])
                nrm = fin.tile([128, HF], fp32, tag="nrm")
                nc.vector.tensor_tensor(
                    out=nrm[:, :], in0=acc[:, 0:HF],
                    in1=rec[:].unsqueeze(2).broadcast_to([128, H, F_OUT]),
                    op=OP.mult)
                # += deg * h_own (fp32, exact skip term)
                nc.vector.scalar_tensor_tensor(
                    out=nrm[:, :], in0=own_sb[:, bb, 0:HF],
                    scalar=own_sb[:, bb, SW:SW + 1],
                    in1=nrm[:, :], op0=OP.mult, op1=OP.add)
                # ELU = max(x,0) + exp(min(x,0)) - 1
                neg = fin.tile([128, HF], fp32, tag="neg")
                nc.vector.tensor_scalar_min(out=neg[:, :], in0=nrm[:, :], scalar1=0.0)
                nc.scalar.activation(neg[:, :], neg[:, :], AF.Exp)
                pos = fin.tile([128, HF], fp32, tag="pos")
                nc.vector.tensor_scalar_max(out=pos[:, :], in0=nrm[:, :], scalar1=0.0)
                res = fin.tile([128, HF], fp32, tag="res")
                nc.vector.scalar_tensor_tensor(
                    out=res[:, :], in0=neg[:, :], scalar=-1.0, in1=pos[:, :],
                    op0=OP.add, op1=OP.add)
                nc.scalar.dma_start(out_d[base_row:base_row + nrows, :],
                                    res[:nrows, :])

            qn = 0
            for bb in range(NBLK):
                qn = emit_main(bb, qn)
                if bb > 0:
                    emit_tail(bb - 1)
            emit_tail(NBLK - 1)

    nc.compile()

    in_maps = []
    for c in range(N_CORES):
        in_maps.append({
            "nft": _pad_cols(NFT, NPAD), "wt": WT.astype(bfnp),
            "m12": M12.astype(bfnp), "brep": b_rep, "iota": iota_rep,
            "ident": ident, "degw": deg_wrap,
            "idx16": idx16_np[c], "rowid": rowid_bf[c],
            "nft_own": nft_own_np[c], "deg_own": deg_own_np[c],
        })
    import os
    trace = bool(os.environ.get("GAT_TRACE"))
    if trace:
        _install_ntff_hook()
    res = run_bass_kernel_spmd(nc, in_maps, list(range(N_CORES)), trace=trace)
    global _last_results
    _last_results = res
    out = np.concatenate([res.results[c]["out"] for c in range(N_CORES)], axis=0)
    return out


def _pad_cols(arr, cols):
    if arr.shape[1] == cols:
        return arr
    out = np.zeros((arr.shape[0], cols), dtype=arr.dtype)
    out[:, :arr.shape[1]] = arr
    return out


# revision 15
# speedup vs baseline: 1.4316x; 1.1531x over previous
"""Multi-head GAT layer on 8 Trainium2 NeuronCores (Bass/Tile SPMD kernel).

Strategy (edge-parallel, target-sharded):
  - Edges sorted by target, sharded across 8 cores by contiguous target
    ranges (N/8 nodes each): softmax + aggregation are core-local.
  - Phase 1a (replicated on every core): one bf16 PE pass over the node
    features builds an augmented per-node table row
      [ h (128) | s2 (8) | s1 (8) | deg (1) | pad ]  (bf16, 512B rows)
    where h = NF @ W.T + b and s1/s2 are the per-node attention scores
    h . a1 / h . a2 (fused into the same matmul via W.T @ A12).
  - Phase 1b (per-core data, same program): the core's own 6250 target
    rows are recomputed into a resident SBUF table (fp32) so phase 2
    needs no self-row gather at all.
  - Phase 2: per 128-target block, edge slots (padded to 128-slot tiles,
    sorted by src) are fetched with dma_gather (int16 indices + a static
    per-group base, 4 SWDGE queues round-robin, 64KB descriptor rings so
    the Q7 never stalls on ring space).  Per tile, a one-hot matrix maps
    slots to local target rows; a single PE matmul per tile accumulates
    the weighted message sum and the softmax denominator together in
    PSUM ([Msg | ex] combo rhs).  The softmax division is pulled out of
    the edge loop (out = acc/denom); the skip term deg * h_i is added
    after the division from the SBUF-resident own-row table.  ELU
    finalize in fp32; contiguous output writes.
"""

import numpy as np

N_CORES = 8
_last_results = None  # BassKernelResults of the most recent run (for harnesses)


def _install_ntff_hook():
    """Register the axon NTFF profiling hook if the image lacks antenv.axon_hooks."""
    import sys, types
    try:
        from antenv.axon_hooks import get_axon_ntff_profile_hook  # noqa: F401
        return
    except ImportError:
        pass
    try:
        mod = types.ModuleType("antenv.axon_hooks")
        holder = [None]
        mod.set_axon_ntff_profile_hook = lambda h: holder.__setitem__(0, h)
        mod.get_axon_ntff_profile_hook = lambda: holder[0]
        sys.modules["antenv.axon_hooks"] = mod
        from trn_agent_boot.trn_boot import _ntff_profile_via_ctypes
        mod.set_axon_ntff_profile_hook(
            _ntff_profile_via_ctypes("/opt/axon/libaxon_pjrt.so"))
    except Exception:
        sys.modules.pop("antenv.axon_hooks", None)


def kernel(node_features, edge_index, W, b, a):
    return gat_multicore(
        np.asarray(node_features, dtype=np.float32),
        np.asarray(edge_index, dtype=np.int32),
        np.asarray(W, dtype=np.float32),
        np.asarray(b, dtype=np.float32),
        np.asarray(a, dtype=np.float32),
    )


def gat_multicore(nf, ei, W, b, a, slope=0.2):
    import sys
    if "/opt/trn_rl_repo" not in sys.path:
        sys.path.insert(0, "/opt/trn_rl_repo")
    import ml_dtypes
    import concourse.bacc as bacc
    import concourse.tile as tile
    import concourse.mybir as mybir
    from concourse import library_config
    from concourse.bass_utils import run_bass_kernel_spmd
    from contextlib import ExitStack

    fp32 = mybir.dt.float32
    bf16 = mybir.dt.bfloat16
    i16 = mybir.dt.int16
    AF = mybir.ActivationFunctionType
    OP = mybir.AluOpType
    bfnp = ml_dtypes.bfloat16

    N, F_IN = nf.shape
    E = ei.shape[1]
    HF = W.shape[0]               # H * F_OUT
    F_OUT = a.shape[0] // 2
    H = HF // F_OUT
    assert F_IN == 128 and HF == 128, "kernel assumes 128 in/out features"
    assert N % N_CORES == 0
    NPC = N // N_CORES            # targets per core
    NBLK = (NPC + 127) // 128     # 128-target blocks per core
    GRP = 8                       # max tiles per gather group
    ROW = 256                     # bf16 elements per table row (512 B)
    SPAN = 30000                  # max int16 index span per gather group

    # ---------------- host prep: weights ----------------
    WT = np.ascontiguousarray(W.T)                       # [F_IN, HF]
    # A12 column order: [s2 (a2) | s1 (a1)] to match the table row layout
    A12 = np.zeros((HF, 2 * H), dtype=np.float32)
    for hd in range(H):
        A12[hd * F_OUT:(hd + 1) * F_OUT, hd] = a[F_OUT:]        # s2
        A12[hd * F_OUT:(hd + 1) * F_OUT, H + hd] = a[:F_OUT]    # s1
    M12 = (WT @ A12).astype(np.float32)                  # [F_IN, 2H]
    b12 = (b @ A12).astype(np.float32)                   # [2H]
    b_ext = np.concatenate([b, b12]).astype(np.float32)  # [144]
    b_rep = np.broadcast_to(b_ext, (128, HF + 2 * H)).copy()
    NFT = np.ascontiguousarray(nf.T).astype(bfnp)        # [F_IN, N] bf16
    iota_rep = np.broadcast_to(
        np.arange(128, dtype=np.float32), (128, 128)).astype(bfnp).copy()
    ident = np.eye(128, dtype=np.float32).astype(bfnp)

    # ---------------- host prep: graph structure ----------------
    src, tgt = ei[0].astype(np.int64), ei[1].astype(np.int64)
    order = np.argsort(tgt, kind="stable")
    ssrc, stgt = src[order], tgt[order]
    deg_full = np.bincount(tgt, minlength=N).astype(np.float32)
    n_nt = (N + 127) // 128
    NPAD = n_nt * 128             # h_tab rows incl. zero padding
    deg_pad = np.zeros(NPAD, dtype=np.float32)
    deg_pad[:N] = deg_full
    deg_wrap = deg_pad.reshape(n_nt, 128).T.astype(bfnp).copy()

    blk_bounds = []
    for c in range(N_CORES):
        bounds = [c * NPC + bb * 128 for bb in range(NBLK)] + [(c + 1) * NPC]
        blk_bounds.append(np.searchsorted(stgt, bounds))
    cnt = np.array([[blk_bounds[c][bb + 1] - blk_bounds[c][bb]
                     for bb in range(NBLK)] for c in range(N_CORES)])
    # edge tiles per block (uniform across cores)
    n_tiles_blk = np.maximum(1, (cnt.max(axis=0) + 127) // 128)
    NT = int(n_tiles_blk.sum())
    t_ofs_blk = np.concatenate([[0], np.cumsum(n_tiles_blk)]).astype(int)

    # Per-core slot arrays; tile t slot p = slot index t*128+p of the block.
    srcs_all = np.zeros((N_CORES, 128, NT), dtype=np.int64)
    rowid_np = np.full((N_CORES, 128, NT), -1.0, dtype=np.float32)
    for c in range(N_CORES):
        for bb in range(NBLK):
            lo, hi = blk_bounds[c][bb], blk_bounds[c][bb + 1]
            nslot = hi - lo
            base_node = c * NPC + bb * 128
            t0 = int(t_ofs_blk[bb])
            net = int(n_tiles_blk[bb])
            ne = net * 128
            if nslot > 0:
                o2 = np.argsort(ssrc[lo:hi], kind="stable")
                s_blk = ssrc[lo:hi][o2]
                pad_val = int(s_blk[-1])
                fl_s = np.full(ne, pad_val, dtype=np.int64)
                fl_r = np.full(ne, -1.0, dtype=np.float32)
                fl_s[:nslot] = s_blk
                fl_r[:nslot] = (stgt[lo:hi][o2] - base_node).astype(np.float32)
                srcs_all[c, :, t0:t0 + net] = fl_s.reshape(net, 128).T
                rowid_np[c, :, t0:t0 + net] = fl_r.reshape(net, 128).T
            # else: pad filled below from other cores
    # blocks empty on some core but not others: align pad values to the
    # cross-core span by borrowing any non-empty core's pad value
    for bb in range(NBLK):
        t0 = int(t_ofs_blk[bb])
        net = int(n_tiles_blk[bb])
        nonempty = [c for c in range(N_CORES) if cnt[c][bb] > 0]
        if nonempty and len(nonempty) < N_CORES:
            ref = int(srcs_all[nonempty[0], 0, t0])
            for c in range(N_CORES):
                if cnt[c][bb] == 0:
                    srcs_all[c, :, t0:t0 + net] = ref

    # Gather groups: consecutive tiles of one block, <= GRP tiles,
    # cross-core index span <= SPAN.  Base per group is the cross-core
    # min -> static program constant.
    groups = []          # (block, tile_lo, n_tiles, base)
    for bb in range(NBLK):
        net = int(n_tiles_blk[bb])
        t0 = int(t_ofs_blk[bb])
        t = 0
        while t < net:
            best = 1
            for w in range(2, min(GRP, net - t) + 1):
                sl = srcs_all[:, :, t0 + t:t0 + t + w]
                if sl.max() - sl.min() > SPAN:
                    break
                best = w
            sl = srcs_all[:, :, t0 + t:t0 + t + best]
            assert sl.max() - sl.min() <= 32000, "single tile span too large"
            groups.append((bb, t, best, int(sl.min())))
            t += best
    groups_by_block = [[] for _ in range(NBLK)]
    for gi, g in enumerate(groups):
        groups_by_block[g[0]].append((gi,) + g[1:])

    g_cols = [(g[2] * 128) // 16 for g in groups]
    g_col_ofs = np.concatenate([[0], np.cumsum(g_cols)]).astype(int)
    IDXC = int(g_col_ofs[-1])
    idx16_np = np.zeros((N_CORES, 128, IDXC), dtype=np.int16)
    for c in range(N_CORES):
        for gi, (bb, tl, w, base) in enumerate(groups):
            t0 = int(t_ofs_blk[bb]) + tl
            rel = (srcs_all[c, :, t0:t0 + w] - base).astype(np.int16)  # [128, w]
            flat = rel.T.reshape(-1)                 # slot order t*128+p
            wrapped = flat.reshape(-1, 16).T         # [16, w*128/16]
            idx16_np[c, :, g_col_ofs[gi]:g_col_ofs[gi + 1]] = np.tile(wrapped, (8, 1))

    rowid_bf = rowid_np.astype(bfnp)

    # Per-core own-node inputs for phase 1b (the core's target rows)
    OWNW = NBLK * 128
    nft_own_np = np.zeros((N_CORES, 128, OWNW), dtype=bfnp)
    deg_own_np = np.zeros((N_CORES, 128, NBLK), dtype=bfnp)
    for c in range(N_CORES):
        nft_own_np[c, :, :NPC] = NFT[:, c * NPC:(c + 1) * NPC]
        dcol = np.zeros(OWNW, dtype=np.float32)
        dcol[:NPC] = deg_full[c * NPC:(c + 1) * NPC]
        deg_own_np[c] = dcol.reshape(NBLK, 128).T.astype(bfnp)

    # ---------------- build the SPMD program ----------------
    # 64 KiB SWDGE scratch -> 4096-descriptor rings per queue: a full-block
    # 1024-idx gather no longer fills the ring, so the Q7 never stalls in
    # await_space waiting for the previous gather on its queue to drain.
    nc = bacc.Bacc("TRN2", target_bir_lowering=False, debug=False,
                   num_devices=N_CORES, num_swdge_queues=4,
                   dynamic_dma_scratch_size=65536)

    nft_d = nc.dram_tensor("nft", [128, NPAD], bf16, kind="ExternalInput").ap()
    wt_d = nc.dram_tensor("wt", [128, HF], bf16, kind="ExternalInput").ap()
    m12_d = nc.dram_tensor("m12", [128, 2 * H], bf16, kind="ExternalInput").ap()
    brep_d = nc.dram_tensor("brep", [128, HF + 2 * H], fp32, kind="ExternalInput").ap()
    iota_d = nc.dram_tensor("iota", [128, 128], bf16, kind="ExternalInput").ap()
    ident_d = nc.dram_tensor("ident", [128, 128], bf16, kind="ExternalInput").ap()
    degw_d = nc.dram_tensor("degw", [128, n_nt], bf16, kind="ExternalInput").ap()
    idx16_d = nc.dram_tensor("idx16", [128, IDXC], i16, kind="ExternalInput").ap()
    rowid_d = nc.dram_tensor("rowid", [128, NT], bf16, kind="ExternalInput").ap()
    nfto_d = nc.dram_tensor("nft_own", [128, OWNW], bf16, kind="ExternalInput").ap()
    dego_d = nc.dram_tensor("deg_own", [128, NBLK], bf16, kind="ExternalInput").ap()

    h_tab = nc.dram_tensor("h_tab", [NPAD, ROW], bf16).ap()
    out_d = nc.dram_tensor("out", [NPC, HF], fp32, kind="ExternalOutput").ap()

    CW = HF + H       # 136: [Msg | ex] combo width
    SW = HF + 2 * H   # 144: phase-1 psum width
    OSW = SW + 1      # own-row width incl. deg
    MAXT = int(n_tiles_blk.max())

    with tile.TileContext(nc) as tc:
        with ExitStack() as ctx:
            cpool = ctx.enter_context(tc.tile_pool(name="consts", bufs=1))
            p1 = ctx.enter_context(tc.tile_pool(name="p1", bufs=3))
            p1ps = ctx.enter_context(tc.tile_pool(name="p1ps", bufs=2, space="PSUM"))
            gp = ctx.enter_context(tc.tile_pool(name="gather", bufs=3))
            mp = ctx.enter_context(tc.tile_pool(name="meta", bufs=3))
            ps_acc = ctx.enter_context(tc.tile_pool(name="ps_acc", bufs=2, space="PSUM"))
            ps_z = ctx.enter_context(tc.tile_pool(name="ps_z", bufs=2, space="PSUM"))
            ps_t = ctx.enter_context(tc.tile_pool(name="ps_t", bufs=1, space="PSUM"))
            fin = ctx.enter_context(tc.tile_pool(name="fin", bufs=4))

            nc.gpsimd.load_library(library_config.mlp)

            wt_sb = cpool.tile([128, HF], bf16)
            nc.sync.dma_start(wt_sb[:], wt_d[:])
            m12_sb = cpool.tile([128, 2 * H], bf16)
            nc.sync.dma_start(m12_sb[:], m12_d[:])
            brep_sb = cpool.tile([128, SW], fp32)
            nc.sync.dma_start(brep_sb[:], brep_d[:])
            iota_sb = cpool.tile([128, 128], bf16)
            nc.sync.dma_start(iota_sb[:], iota_d[:])
            ident_sb = cpool.tile([128, 128], bf16)
            nc.sync.dma_start(ident_sb[:], ident_d[:])
            idx_sb = cpool.tile([128, IDXC], i16)
            nc.sync.dma_start(idx_sb[:], idx16_d[:])
            rid_all = cpool.tile([128, NT], bf16)
            nc.sync.dma_start(rid_all[:], rowid_d[:])
            dego_sb = cpool.tile([128, NBLK], bf16)
            nc.sync.dma_start(dego_sb[:], dego_d[:])
            # SBUF-resident own-row table [h | s2 | s1 | deg] fp32
            own_sb = cpool.tile([128, NBLK, OSW], fp32)

            b_is_zero = not np.any(b_ext)

            # ---------- phase 1b: own rows -> resident SBUF table ----------
            for ob0 in range(0, NBLK, 2):
                nk = min(2, NBLK - ob0)
                nfo = p1.tile([128, 256], bf16, tag="nfo")
                nc.sync.dma_start(nfo[:, :nk * 128],
                                  nfto_d[:, ob0 * 128:ob0 * 128 + nk * 128])
                ps = p1ps.tile([128, 2, SW], fp32, space="PSUM", tag="p1ps")
                for k in range(nk):
                    nc.tensor.matmul(ps[:, k, 0:HF],
                                     lhsT=nfo[:, k * 128:(k + 1) * 128],
                                     rhs=wt_sb[:], start=True, stop=True)
                    nc.tensor.matmul(ps[:, k, HF:SW],
                                     lhsT=nfo[:, k * 128:(k + 1) * 128],
                                     rhs=m12_sb[:], start=True, stop=True)
                if b_is_zero:
                    nc.vector.tensor_copy(own_sb[:, ob0:ob0 + nk, 0:SW],
                                          ps[:, :nk, :])
                else:
                    nc.vector.tensor_tensor(
                        out=own_sb[:, ob0:ob0 + nk, 0:SW], in0=ps[:, :nk, :],
                        in1=brep_sb[:].unsqueeze(1).broadcast_to([128, nk, SW]),
                        op=OP.add)
                nc.vector.tensor_copy(own_sb[:, ob0:ob0 + nk, SW:SW + 1],
                                      dego_sb[:, ob0:ob0 + nk].unsqueeze(2))

            # ---------- phase 1a: full h table (replicated) ----------
            # Loops over NPAD (zero-padded nodes) so padded rows are finite.
            CH = 512
            for j0 in range(0, NPAD, CH):
                w = min(CH, NPAD - j0)
                nfc = p1.tile([128, CH], bf16, tag="nfc")
                nc.sync.dma_start(nfc[:, :w], nft_d[:, j0:j0 + w])
                ncols = (w + 127) // 128
                degc = p1.tile([128, CH // 128], bf16, tag="degc")
                nc.sync.dma_start(degc[:, :ncols],
                                  degw_d[:, j0 // 128:j0 // 128 + ncols])
                for k0 in range(0, w, 256):
                    # two node-tiles per hrow buffer / table write
                    kw2 = min(256, w - k0)
                    hrow = p1.tile([128, 2, ROW], bf16, tag="hrow")
                    nk = (kw2 + 127) // 128
                    ps = p1ps.tile([128, 2, SW], fp32, space="PSUM", tag="p1ps")
                    for k in range(nk):
                        kk = k0 + k * 128
                        kw = min(128, w - kk)
                        nc.tensor.matmul(ps[:kw, k, 0:HF],
                                         lhsT=nfc[:, kk:kk + kw],
                                         rhs=wt_sb[:], start=True, stop=True)
                        nc.tensor.matmul(ps[:kw, k, HF:SW],
                                         lhsT=nfc[:, kk:kk + kw],
                                         rhs=m12_sb[:], start=True, stop=True)
                    # alternate the PSUM->SBUF cast between DVE and ACT so
                    # neither engine serializes phase 1
                    use_act = (j0 // CH + k0 // 256) % 2 == 1
                    if b_is_zero and use_act:
                        nc.scalar.copy(hrow[:, :nk, 0:SW], ps[:, :nk, :])
                    elif b_is_zero:
                        nc.vector.tensor_copy(hrow[:, :nk, 0:SW], ps[:, :nk, :])
                    else:
                        nc.vector.tensor_tensor(
                            out=hrow[:, :nk, 0:SW], in0=ps[:, :nk, :],
                            in1=brep_sb[:].unsqueeze(1).broadcast_to([128, nk, SW]),
                            op=OP.add)
                    nc.vector.tensor_copy(
                        hrow[:, :nk, SW:SW + 1],
                        degc[:, k0 // 128:k0 // 128 + nk].unsqueeze(2))
                    n0 = j0 + k0
                    nc.scalar.dma_start(
                        h_tab[n0:n0 + kw2, :].rearrange(
                            "(k p) r -> p k r", k=nk),
                        hrow[:, :nk, :])

            # ---------- phase 2: edge processing ----------
            # Per block: main = gathers + scores + weighted-agg matmuls into
            # PSUM; tail = softmax division + skip + ELU + output write.
            # Tails are emitted one block late so their PSUM waits overlap
            # with the next block's compute.
            blk_state = {}

            def emit_main(bb, qn0):
                net = int(n_tiles_blk[bb])
                t0 = int(t_ofs_blk[bb])
                qn = qn0

                acc = ps_acc.tile([128, CW], fp32, space="PSUM", tag="acc")
                G = gp.tile([128, MAXT, ROW], bf16, tag="G")
                me = gp.tile([128, MAXT, CW], bf16, tag="me")
                oh = gp.tile([128, MAXT, 128], bf16, tag="oh")

                for gi, tl, wdt, base in groups_by_block[bb]:
                    nc.gpsimd.dma_gather(
                        out_ap=G[:, tl:tl + wdt, :],
                        in_ap=h_tab[base:, :],
                        idxs_ap=idx_sb[:, g_col_ofs[gi]:g_col_ofs[gi + 1]],
                        num_idxs=wdt * 128, num_idxs_reg=wdt * 128,
                        elem_size=ROW, queue_num=qn % 4)
                    qn += 1

                # one-hot (bf16) for the edge tiles, batched
                nc.vector.tensor_tensor(
                    out=oh[:, :net, :],
                    in0=rid_all[:, t0:t0 + net].unsqueeze(2).broadcast_to(
                        [128, net, 128]),
                    in1=iota_sb[:].unsqueeze(1).broadcast_to([128, net, 128]),
                    op=OP.is_equal)

                # s1 of the block's targets (bf16 for the PE matmul rhs)
                s1bf = mp.tile([128, H], bf16, tag="s1bf")
                nc.vector.tensor_copy(s1bf[:], own_sb[:, bb, SW - H:SW])

                # per-tile: transpose(one-hot) -> s1e matmul
                zps = ps_z.tile([128, MAXT, H], fp32, space="PSUM", tag="zps")
                TGRP = 8
                for q0 in range(0, net, TGRP):
                    qw = min(TGRP, net - q0)
                    ohT_ps = ps_t.tile([128, TGRP, 128], bf16, space="PSUM",
                                       tag="ohT_ps")
                    for t in range(q0, q0 + qw):
                        nc.tensor.transpose(ohT_ps[:, t - q0, :], in_=oh[:, t, :],
                                            identity=ident_sb[:])
                    ohT_sb = gp.tile([128, TGRP, 128], bf16, tag="ohT_sb")
                    nc.scalar.copy(ohT_sb[:, :qw, :], ohT_ps[:, :qw, :])
                    for t in range(q0, q0 + qw):
                        nc.tensor.matmul(zps[:, t, :], lhsT=ohT_sb[:, t - q0, :],
                                         rhs=s1bf[:], start=True, stop=True)

                # ex = exp(leakyrelu(s1e + s2)); z = zps + G.s2 on DVE
                z_sb = mp.tile([128, MAXT, H], fp32, tag="z_sb")
                nc.vector.tensor_tensor(out=z_sb[:, :net, :], in0=zps[:, :net, :],
                                        in1=G[:, :net, HF:HF + H], op=OP.add)
                ext = mp.tile([128, MAXT, H], bf16, tag="ext")
                nc.vector.scalar_tensor_tensor(
                    out=ext[:, :net, :], in0=z_sb[:, :net, :], scalar=slope,
                    in1=z_sb[:, :net, :], op0=OP.mult, op1=OP.max)
                nc.scalar.activation(ext[:, :net, :], ext[:, :net, :], AF.Exp)
                # combo rhs tile: [Msg | ex] so acc+denom take one matmul/tile
                nc.vector.tensor_copy(me[:, :net, HF:CW], ext[:, :net, :])
                # expand ex across F_OUT on the ACT engine so the big DVE
                # multiply runs on contiguous operands (2x bf16 rate)
                ex128 = gp.tile([128, MAXT, H, F_OUT], bf16, tag="ex128")
                nc.scalar.copy(
                    ex128[:, :net, :, :],
                    ext[:, :net, :].unsqueeze(3).broadcast_to(
                        [128, net, H, F_OUT]))
                nc.vector.tensor_tensor(
                    out=me[:, 0:net, 0:HF], in0=G[:, 0:net, 0:HF],
                    in1=ex128[:, 0:net, :, :], op=OP.mult)

                for t in range(net):
                    nc.tensor.matmul(acc[:, :], lhsT=oh[:, t, :],
                                     rhs=me[:, t, :],
                                     start=(t == 0), stop=(t == net - 1))

                blk_state[bb] = acc
                return qn

            def emit_tail(bb):
                base_row = bb * 128
                nrows = min(128, NPC - base_row)
                acc = blk_state.pop(bb)

                rec = fin.tile([128, H], fp32, tag="rec")
                nc.vector.tensor_scalar_add(out=rec[:, :], in0=acc[:, HF:CW],
                                            scalar1=1e-30)
                nc.vector.reciprocal(rec[:, :], rec[:, :])
                nrm = fin.tile([128, HF], fp32, tag="nrm")
                nc.vector.tensor_tensor(
                    out=nrm[:, :], in0=acc[:, 0:HF],
                    in1=rec[:].unsqueeze(2).broadcast_to([128, H, F_OUT]),
                    op=OP.mult)
                # += deg * h_own (fp32, exact skip term)
                nc.vector.scalar_tensor_tensor(
                    out=nrm[:, :], in0=own_sb[:, bb, 0:HF],
                    scalar=own_sb[:, bb, SW:SW + 1],
                    in1=nrm[:, :], op0=OP.mult, op1=OP.add)
                # ELU = max(x, exp(min(x,0)) - 1)
                neg = fin.tile([128, HF], fp32, tag="neg")
                nc.vector.tensor_scalar_min(out=neg[:, :], in0=nrm[:, :], scalar1=0.0)
                nc.scalar.activation(neg[:, :], neg[:, :], AF.Exp)
                res = fin.tile([128, HF], fp32, tag="res")
                nc.vector.scalar_tensor_tensor(
                    out=res[:, :], in0=neg[:, :], scalar=-1.0, in1=nrm[:, :],
                    op0=OP.add, op1=OP.max)
                nc.scalar.dma_start(out_d[base_row:base_row + nrows, :],
                                    res[:nrows, :])

            qn = 0
            for bb in range(NBLK):
                qn = emit_main(bb, qn)
                if bb > 0:
                    emit_tail(bb - 1)
            emit_tail(NBLK - 1)

    nc.compile()

    in_maps = []
    for c in range(N_CORES):
        in_maps.append({
            "nft": _pad_cols(NFT, NPAD), "wt": WT.astype(bfnp),
            "m12": M12.astype(bfnp), "brep": b_rep, "iota": iota_rep,
            "ident": ident, "degw": deg_wrap,
            "idx16": idx16_np[c], "rowid": rowid_bf[c],
            "nft_own": nft_own_np[c], "deg_own": deg_own_np[c],
        })
    import os
    trace = bool(os.environ.get("GAT_TRACE"))
    if trace:
        _install_ntff_hook()
    res = run_bass_kernel_spmd(nc, in_maps, list(range(N_CORES)), trace=trace)
    global _last_results
    _last_results = res
    out = np.concatenate([res.results[c]["out"] for c in range(N_CORES)], axis=0)
    return out


def _pad_cols(arr, cols):
    if arr.shape[1] == cols:
        return arr
    out = np.zeros((arr.shape[0], cols), dtype=arr.dtype)
    out[:, :arr.shape[1]] = arr
    return out


# revision 16
# speedup vs baseline: 1.5022x; 1.0493x over previous
"""Multi-head GAT layer on 8 Trainium2 NeuronCores (Bass/Tile SPMD kernel).

Strategy (edge-parallel, target-sharded):
  - Edges sorted by target, sharded across 8 cores by contiguous target
    ranges (N/8 nodes each): softmax + aggregation are core-local.
  - Phase 1a (replicated on every core): one bf16 PE pass over the node
    features builds a per-node table row [ h (128) | s2 (8) | s1 (8) ]
    (bf16, 512B rows) where h = NF @ W.T + b and s1/s2 are the per-node
    attention scores h . a1 / h . a2 (fused into the same matmul via
    W.T @ A12).
  - Phase 1b (per-core data, same program): the core's own 6250 target
    rows are recomputed into a resident SBUF table (fp32, including
    degree) so phase 2 needs no self-row gather at all.
  - Phase 2, software-pipelined per 128-target block:
      A: edge slots (padded to 128-slot tiles, sorted by src) fetched
         with dma_gather (int16 indices + static per-group base, 4 SWDGE
         queues, 64KB descriptor rings);
      B: slot->target one-hots built on DVE: oh from the resident rowid
         table, ohT from a host-uploaded free-axis rowid pattern (int8)
         so no PE transposes are needed;
      C: s1-per-slot via small PE matmuls against ohT;
      D: scores z = s1e+s2, ex = exp(lrelu(z)) (DVE+ACT), ex expanded
         across F_OUT on ACT so the big DVE multiply runs contiguous;
         a single PE matmul per tile accumulates [Msg | ex] into PSUM;
      E: tail = softmax division, skip term from the SBUF own-table,
         ELU as max(x, exp(min(x,0))-1), contiguous output write.
    Stages are emitted skewed (A/B/C for block i, D for i-1, E for i-2)
    so each in-order engine queue interleaves independent blocks.
"""

import numpy as np

N_CORES = 8
_last_results = None  # BassKernelResults of the most recent run (for harnesses)


def _install_ntff_hook():
    """Register the axon NTFF profiling hook if the image lacks antenv.axon_hooks."""
    import sys, types
    try:
        from antenv.axon_hooks import get_axon_ntff_profile_hook  # noqa: F401
        return
    except ImportError:
        pass
    try:
        mod = types.ModuleType("antenv.axon_hooks")
        holder = [None]
        mod.set_axon_ntff_profile_hook = lambda h: holder.__setitem__(0, h)
        mod.get_axon_ntff_profile_hook = lambda: holder[0]
        sys.modules["antenv.axon_hooks"] = mod
        from trn_agent_boot.trn_boot import _ntff_profile_via_ctypes
        mod.set_axon_ntff_profile_hook(
            _ntff_profile_via_ctypes("/opt/axon/libaxon_pjrt.so"))
    except Exception:
        sys.modules.pop("antenv.axon_hooks", None)


def kernel(node_features, edge_index, W, b, a):
    return gat_multicore(
        np.asarray(node_features, dtype=np.float32),
        np.asarray(edge_index, dtype=np.int32),
        np.asarray(W, dtype=np.float32),
        np.asarray(b, dtype=np.float32),
        np.asarray(a, dtype=np.float32),
    )


def gat_multicore(nf, ei, W, b, a, slope=0.2):
    import sys
    if "/opt/trn_rl_repo" not in sys.path:
        sys.path.insert(0, "/opt/trn_rl_repo")
    import ml_dtypes
    import concourse.bacc as bacc
    import concourse.tile as tile
    import concourse.mybir as mybir
    from concourse import library_config
    from concourse.bass_utils import run_bass_kernel_spmd
    from contextlib import ExitStack

    fp32 = mybir.dt.float32
    bf16 = mybir.dt.bfloat16
    i16 = mybir.dt.int16
    i8 = mybir.dt.int8
    AF = mybir.ActivationFunctionType
    OP = mybir.AluOpType
    bfnp = ml_dtypes.bfloat16

    N, F_IN = nf.shape
    E = ei.shape[1]
    HF = W.shape[0]               # H * F_OUT
    F_OUT = a.shape[0] // 2
    H = HF // F_OUT
    assert F_IN == 128 and HF == 128, "kernel assumes 128 in/out features"
    assert N % N_CORES == 0
    NPC = N // N_CORES            # targets per core
    NBLK = (NPC + 127) // 128     # 128-target blocks per core
    GRP = 8                       # max tiles per gather group
    ROW = 256                     # bf16 elements per table row (512 B)
    SPAN = 30000                  # max int16 index span per gather group

    # ---------------- host prep: weights ----------------
    WT = np.ascontiguousarray(W.T)                       # [F_IN, HF]
    # A12 column order: [s2 (a2) | s1 (a1)] to match the table row layout
    A12 = np.zeros((HF, 2 * H), dtype=np.float32)
    for hd in range(H):
        A12[hd * F_OUT:(hd + 1) * F_OUT, hd] = a[F_OUT:]        # s2
        A12[hd * F_OUT:(hd + 1) * F_OUT, H + hd] = a[:F_OUT]    # s1
    M12 = (WT @ A12).astype(np.float32)                  # [F_IN, 2H]
    b12 = (b @ A12).astype(np.float32)                   # [2H]
    b_ext = np.concatenate([b, b12]).astype(np.float32)  # [144]
    b_rep = np.broadcast_to(b_ext, (128, HF + 2 * H)).copy()
    NFT = np.ascontiguousarray(nf.T).astype(bfnp)        # [F_IN, N] bf16
    iota_rep = np.broadcast_to(
        np.arange(128, dtype=np.float32), (128, 128)).astype(bfnp).copy()
    # per-partition index constant, replicated along the free axis (int8)
    iotaP_i8 = np.broadcast_to(
        np.arange(128, dtype=np.int8)[:, None], (128, 128)).copy()

    # ---------------- host prep: graph structure ----------------
    src, tgt = ei[0].astype(np.int64), ei[1].astype(np.int64)
    order = np.argsort(tgt, kind="stable")
    ssrc, stgt = src[order], tgt[order]
    deg_full = np.bincount(tgt, minlength=N).astype(np.float32)
    n_nt = (N + 127) // 128
    NPAD = n_nt * 128             # h_tab rows incl. zero padding

    blk_bounds = []
    for c in range(N_CORES):
        bounds = [c * NPC + bb * 128 for bb in range(NBLK)] + [(c + 1) * NPC]
        blk_bounds.append(np.searchsorted(stgt, bounds))
    cnt = np.array([[blk_bounds[c][bb + 1] - blk_bounds[c][bb]
                     for bb in range(NBLK)] for c in range(N_CORES)])
    # edge tiles per block (uniform across cores)
    n_tiles_blk = np.maximum(1, (cnt.max(axis=0) + 127) // 128)
    NT = int(n_tiles_blk.sum())
    t_ofs_blk = np.concatenate([[0], np.cumsum(n_tiles_blk)]).astype(int)

    # Per-core slot arrays; tile t slot p = slot index t*128+p of the block.
    srcs_all = np.zeros((N_CORES, 128, NT), dtype=np.int64)
    rowid_np = np.full((N_CORES, 128, NT), -1.0, dtype=np.float32)
    for c in range(N_CORES):
        for bb in range(NBLK):
            lo, hi = blk_bounds[c][bb], blk_bounds[c][bb + 1]
            nslot = hi - lo
            base_node = c * NPC + bb * 128
            t0 = int(t_ofs_blk[bb])
            net = int(n_tiles_blk[bb])
            ne = net * 128
            if nslot > 0:
                o2 = np.argsort(ssrc[lo:hi], kind="stable")
                s_blk = ssrc[lo:hi][o2]
                pad_val = int(s_blk[-1])
                fl_s = np.full(ne, pad_val, dtype=np.int64)
                fl_r = np.full(ne, -1.0, dtype=np.float32)
                fl_s[:nslot] = s_blk
                fl_r[:nslot] = (stgt[lo:hi][o2] - base_node).astype(np.float32)
                srcs_all[c, :, t0:t0 + net] = fl_s.reshape(net, 128).T
                rowid_np[c, :, t0:t0 + net] = fl_r.reshape(net, 128).T
            # else: pad filled below from other cores
    for bb in range(NBLK):
        t0 = int(t_ofs_blk[bb])
        net = int(n_tiles_blk[bb])
        nonempty = [c for c in range(N_CORES) if cnt[c][bb] > 0]
        if nonempty and len(nonempty) < N_CORES:
            ref = int(srcs_all[nonempty[0], 0, t0])
            for c in range(N_CORES):
                if cnt[c][bb] == 0:
                    srcs_all[c, :, t0:t0 + net] = ref

    # Gather groups: consecutive tiles of one block, <= GRP tiles,
    # cross-core index span <= SPAN.
    groups = []          # (block, tile_lo, n_tiles, base)
    for bb in range(NBLK):
        net = int(n_tiles_blk[bb])
        t0 = int(t_ofs_blk[bb])
        t = 0
        while t < net:
            best = 1
            for w in range(2, min(GRP, net - t) + 1):
                sl = srcs_all[:, :, t0 + t:t0 + t + w]
                if sl.max() - sl.min() > SPAN:
                    break
                best = w
            sl = srcs_all[:, :, t0 + t:t0 + t + best]
            assert sl.max() - sl.min() <= 32000, "single tile span too large"
            groups.append((bb, t, best, int(sl.min())))
            t += best
    groups_by_block = [[] for _ in range(NBLK)]
    for gi, g in enumerate(groups):
        groups_by_block[g[0]].append((gi,) + g[1:])

    g_cols = [(g[2] * 128) // 16 for g in groups]
    g_col_ofs = np.concatenate([[0], np.cumsum(g_cols)]).astype(int)
    IDXC = int(g_col_ofs[-1])
    idx16_np = np.zeros((N_CORES, 128, IDXC), dtype=np.int16)
    for c in range(N_CORES):
        for gi, (bb, tl, w, base) in enumerate(groups):
            t0 = int(t_ofs_blk[bb]) + tl
            rel = (srcs_all[c, :, t0:t0 + w] - base).astype(np.int16)  # [128, w]
            flat = rel.T.reshape(-1)                 # slot order t*128+p
            wrapped = flat.reshape(-1, 16).T         # [16, w*128/16]
            idx16_np[c, :, g_col_ofs[gi]:g_col_ofs[gi + 1]] = np.tile(wrapped, (8, 1))

    rowid_bf = rowid_np.astype(bfnp)
    # free-axis rowid pattern for the transposed one-hot (int8, replicated
    # across partitions): rowT[p, t*128+slot] = rowid[slot, t]
    rowT_np = np.empty((N_CORES, 128, NT * 128), dtype=np.int8)
    for c in range(N_CORES):
        flat = rowid_np[c].T.reshape(-1).astype(np.int8)     # [(t,slot)]
        rowT_np[c] = np.broadcast_to(flat, (128, NT * 128))

    # Per-core own-node inputs for phase 1b (the core's target rows)
    OWNW = NBLK * 128
    nft_own_np = np.zeros((N_CORES, 128, OWNW), dtype=bfnp)
    deg_own_np = np.zeros((N_CORES, 128, NBLK), dtype=bfnp)
    for c in range(N_CORES):
        nft_own_np[c, :, :NPC] = NFT[:, c * NPC:(c + 1) * NPC]
        dcol = np.zeros(OWNW, dtype=np.float32)
        dcol[:NPC] = deg_full[c * NPC:(c + 1) * NPC]
        deg_own_np[c] = dcol.reshape(NBLK, 128).T.astype(bfnp)

    # ---------------- build the SPMD program ----------------
    # 48 KiB SWDGE scratch -> 3072-descriptor rings per queue so a 1024-idx
    # gather never fills a ring and the Q7 doesn't stall in await_space.
    nc = bacc.Bacc("TRN2", target_bir_lowering=False, debug=False,
                   num_devices=N_CORES, num_swdge_queues=4,
                   dynamic_dma_scratch_size=49152)

    nft_d = nc.dram_tensor("nft", [128, NPAD], bf16, kind="ExternalInput").ap()
    wt_d = nc.dram_tensor("wt", [128, HF], bf16, kind="ExternalInput").ap()
    m12_d = nc.dram_tensor("m12", [128, 2 * H], bf16, kind="ExternalInput").ap()
    brep_d = nc.dram_tensor("brep", [128, HF + 2 * H], fp32, kind="ExternalInput").ap()
    iota_d = nc.dram_tensor("iota", [128, 128], bf16, kind="ExternalInput").ap()
    iotap_d = nc.dram_tensor("iotap", [128, 128], i8, kind="ExternalInput").ap()
    idx16_d = nc.dram_tensor("idx16", [128, IDXC], i16, kind="ExternalInput").ap()
    rowid_d = nc.dram_tensor("rowid", [128, NT], bf16, kind="ExternalInput").ap()
    rowt_d = nc.dram_tensor("rowt", [128, NT * 128], i8, kind="ExternalInput").ap()
    nfto_d = nc.dram_tensor("nft_own", [128, OWNW], bf16, kind="ExternalInput").ap()
    dego_d = nc.dram_tensor("deg_own", [128, NBLK], bf16, kind="ExternalInput").ap()

    h_tab = nc.dram_tensor("h_tab", [NPAD, ROW], bf16).ap()
    out_d = nc.dram_tensor("out", [NPC, HF], fp32, kind="ExternalOutput").ap()

    CW = HF + H       # 136: [Msg | ex] combo width
    SW = HF + 2 * H   # 144: phase-1 psum width
    OSW = SW + 1      # own-row width incl. deg
    MAXT = int(n_tiles_blk.max())

    with tile.TileContext(nc) as tc:
        with ExitStack() as ctx:
            cpool = ctx.enter_context(tc.tile_pool(name="consts", bufs=1))
            p1 = ctx.enter_context(tc.tile_pool(name="p1", bufs=3))
            p1ps = ctx.enter_context(tc.tile_pool(name="p1ps", bufs=2, space="PSUM"))
            gp = ctx.enter_context(tc.tile_pool(name="gather", bufs=3))
            mp = ctx.enter_context(tc.tile_pool(name="meta", bufs=3))
            ps_acc = ctx.enter_context(tc.tile_pool(name="ps_acc", bufs=2, space="PSUM"))
            ps_z = ctx.enter_context(tc.tile_pool(name="ps_z", bufs=2, space="PSUM"))
            fin = ctx.enter_context(tc.tile_pool(name="fin", bufs=4))

            nc.gpsimd.load_library(library_config.mlp)

            wt_sb = cpool.tile([128, HF], bf16)
            nc.sync.dma_start(wt_sb[:], wt_d[:])
            m12_sb = cpool.tile([128, 2 * H], bf16)
            nc.sync.dma_start(m12_sb[:], m12_d[:])
            brep_sb = cpool.tile([128, SW], fp32)
            nc.sync.dma_start(brep_sb[:], brep_d[:])
            iota_sb = cpool.tile([128, 128], bf16)
            nc.sync.dma_start(iota_sb[:], iota_d[:])
            iotap_sb = cpool.tile([128, 128], i8)
            nc.sync.dma_start(iotap_sb[:], iotap_d[:])
            idx_sb = cpool.tile([128, IDXC], i16)
            nc.sync.dma_start(idx_sb[:], idx16_d[:])
            rid_all = cpool.tile([128, NT], bf16)
            nc.sync.dma_start(rid_all[:], rowid_d[:])
            dego_sb = cpool.tile([128, NBLK], bf16)
            nc.sync.dma_start(dego_sb[:], dego_d[:])
            # SBUF-resident own-row table [h | s2 | s1 | deg] fp32
            own_sb = cpool.tile([128, NBLK, OSW], fp32)

            b_is_zero = not np.any(b_ext)

            # ---------- phase 1b: own rows -> resident SBUF table ----------
            for ob0 in range(0, NBLK, 2):
                nk = min(2, NBLK - ob0)
                nfo = p1.tile([128, 256], bf16, tag="nfo")
                nc.sync.dma_start(nfo[:, :nk * 128],
                                  nfto_d[:, ob0 * 128:ob0 * 128 + nk * 128])
                ps = p1ps.tile([128, 2, SW], fp32, space="PSUM", tag="p1ps")
                for k in range(nk):
                    nc.tensor.matmul(ps[:, k, 0:HF],
                                     lhsT=nfo[:, k * 128:(k + 1) * 128],
                                     rhs=wt_sb[:], start=True, stop=True)
                    nc.tensor.matmul(ps[:, k, HF:SW],
                                     lhsT=nfo[:, k * 128:(k + 1) * 128],
                                     rhs=m12_sb[:], start=True, stop=True)
                if b_is_zero:
                    nc.vector.tensor_copy(own_sb[:, ob0:ob0 + nk, 0:SW],
                                          ps[:, :nk, :])
                else:
                    nc.vector.tensor_tensor(
                        out=own_sb[:, ob0:ob0 + nk, 0:SW], in0=ps[:, :nk, :],
                        in1=brep_sb[:].unsqueeze(1).broadcast_to([128, nk, SW]),
                        op=OP.add)
                nc.vector.tensor_copy(own_sb[:, ob0:ob0 + nk, SW:SW + 1],
                                      dego_sb[:, ob0:ob0 + nk].unsqueeze(2))

            # ---------- phase 1a: full h table (replicated) ----------
            # One 512-node chunk per iteration: 4 matmul pairs into two PSUM
            # tiles, PSUM->SBUF casts alternating DVE/ACT, one table write.
            CH = 512
            for j0 in range(0, NPAD, CH):
                w = min(CH, NPAD - j0)
                nfc = p1.tile([128, CH], bf16, tag="nfc")
                nc.sync.dma_start(nfc[:, :w], nft_d[:, j0:j0 + w])
                nkc = (w + 127) // 128
                hrow = p1.tile([128, 4, ROW], bf16, tag="hrow")
                for k0 in range(0, w, 256):
                    kw2 = min(256, w - k0)
                    nk = (kw2 + 127) // 128
                    ps = p1ps.tile([128, 2, SW], fp32, space="PSUM", tag="p1ps")
                    for k in range(nk):
                        kk = k0 + k * 128
                        nc.tensor.matmul(ps[:, k, 0:HF],
                                         lhsT=nfc[:, kk:kk + 128],
                                         rhs=wt_sb[:], start=True, stop=True)
                        nc.tensor.matmul(ps[:, k, HF:SW],
                                         lhsT=nfc[:, kk:kk + 128],
                                         rhs=m12_sb[:], start=True, stop=True)
                    ko = k0 // 128
                    if b_is_zero and k0 == 0:
                        nc.vector.tensor_copy(hrow[:, ko:ko + nk, 0:SW],
                                              ps[:, :nk, :])
                    elif b_is_zero:
                        nc.scalar.copy(hrow[:, ko:ko + nk, 0:SW], ps[:, :nk, :])
                    else:
                        nc.vector.tensor_tensor(
                            out=hrow[:, ko:ko + nk, 0:SW], in0=ps[:, :nk, :],
                            in1=brep_sb[:].unsqueeze(1).broadcast_to([128, nk, SW]),
                            op=OP.add)
                nc.scalar.dma_start(
                    h_tab[j0:j0 + w, :].rearrange("(k p) r -> p k r", k=nkc),
                    hrow[:, :nkc, :])

            # ---------- phase 2: edge processing (software-pipelined) ----------
            blk_state = {}

            def stage_fetch(bb, qn0):
                """A: dma_gather the block's edge rows; B: build both one-hot
                orientations; C: s1-per-slot matmuls."""
                net = int(n_tiles_blk[bb])
                t0 = int(t_ofs_blk[bb])
                qn = qn0

                G = gp.tile([128, MAXT, ROW], bf16, tag="G")
                for gi, tl, wdt, base in groups_by_block[bb]:
                    nc.gpsimd.dma_gather(
                        out_ap=G[:, tl:tl + wdt, :],
                        in_ap=h_tab[base:, :],
                        idxs_ap=idx_sb[:, g_col_ofs[gi]:g_col_ofs[gi + 1]],
                        num_idxs=wdt * 128, num_idxs_reg=wdt * 128,
                        elem_size=ROW, queue_num=qn % 4)
                    qn += 1

                # oh[slot, t, tgt] (bf16) for the acc matmul lhsT
                oh = gp.tile([128, MAXT, 128], bf16, tag="oh")
                nc.vector.tensor_tensor(
                    out=oh[:, :net, :],
                    in0=rid_all[:, t0:t0 + net].unsqueeze(2).broadcast_to(
                        [128, net, 128]),
                    in1=iota_sb[:].unsqueeze(1).broadcast_to([128, net, 128]),
                    op=OP.is_equal)

                # ohT[tgt, t, slot] (bf16) from the uploaded free-axis rowid
                # pattern -- all-contiguous DVE op, no PE transposes
                rowT = gp.tile([128, MAXT, 128], i8, tag="rowT")
                nc.sync.dma_start(rowT[:, :net, :],
                                  rowt_d[:, t0 * 128:(t0 + net) * 128])
                ohT = gp.tile([128, MAXT, 128], bf16, tag="ohT")
                nc.vector.tensor_tensor(
                    out=ohT[:, :net, :], in0=rowT[:, :net, :],
                    in1=iotap_sb[:].unsqueeze(1).broadcast_to([128, net, 128]),
                    op=OP.is_equal)

                # s1 of the block's targets (bf16 rhs), then s1e per slot
                s1bf = mp.tile([128, H], bf16, tag="s1bf")
                nc.vector.tensor_copy(s1bf[:], own_sb[:, bb, SW - H:SW])
                zps = ps_z.tile([128, MAXT, H], fp32, space="PSUM", tag="zps")
                for t in range(net):
                    nc.tensor.matmul(zps[:, t, :], lhsT=ohT[:, t, :],
                                     rhs=s1bf[:], start=True, stop=True)

                blk_state[bb] = (G, oh, zps, qn0)
                return qn

            def stage_compute(bb):
                """D: scores, weights, weighted-message accumulation."""
                net = int(n_tiles_blk[bb])
                G, oh, zps, _ = blk_state[bb]

                acc = ps_acc.tile([128, CW], fp32, space="PSUM", tag="acc")
                me = gp.tile([128, MAXT, CW], bf16, tag="me")

                z_sb = mp.tile([128, MAXT, H], fp32, tag="z_sb")
                nc.vector.tensor_tensor(out=z_sb[:, :net, :], in0=zps[:, :net, :],
                                        in1=G[:, :net, HF:HF + H], op=OP.add)
                ext = mp.tile([128, MAXT, H], bf16, tag="ext")
                nc.vector.scalar_tensor_tensor(
                    out=ext[:, :net, :], in0=z_sb[:, :net, :], scalar=slope,
                    in1=z_sb[:, :net, :], op0=OP.mult, op1=OP.max)
                nc.scalar.activation(ext[:, :net, :], ext[:, :net, :], AF.Exp)
                nc.vector.tensor_copy(me[:, :net, HF:CW], ext[:, :net, :])
                # expand ex across F_OUT on ACT so the big DVE multiply is
                # contiguous x contiguous (2x bf16 rate)
                ex128 = gp.tile([128, MAXT, H, F_OUT], bf16, tag="ex128")
                nc.scalar.copy(
                    ex128[:, :net, :, :],
                    ext[:, :net, :].unsqueeze(3).broadcast_to(
                        [128, net, H, F_OUT]))
                nc.vector.tensor_tensor(
                    out=me[:, 0:net, 0:HF], in0=G[:, 0:net, 0:HF],
                    in1=ex128[:, 0:net, :, :], op=OP.mult)

                for t in range(net):
                    nc.tensor.matmul(acc[:, :], lhsT=oh[:, t, :],
                                     rhs=me[:, t, :],
                                     start=(t == 0), stop=(t == net - 1))
                blk_state[bb] = acc

            def stage_tail(bb):
                """E: softmax division, skip term, ELU, output write."""
                base_row = bb * 128
                nrows = min(128, NPC - base_row)
                acc = blk_state.pop(bb)

                rec = fin.tile([128, H], fp32, tag="rec")
                nc.vector.tensor_scalar_add(out=rec[:, :], in0=acc[:, HF:CW],
                                            scalar1=1e-30)
                nc.vector.reciprocal(rec[:, :], rec[:, :])
                nrm = fin.tile([128, HF], fp32, tag="nrm")
                nc.vector.tensor_tensor(
                    out=nrm[:, :], in0=acc[:, 0:HF],
                    in1=rec[:].unsqueeze(2).broadcast_to([128, H, F_OUT]),
                    op=OP.mult)
                # += deg * h_own (fp32, exact skip term)
                nc.vector.scalar_tensor_tensor(
                    out=nrm[:, :], in0=own_sb[:, bb, 0:HF],
                    scalar=own_sb[:, bb, SW:SW + 1],
                    in1=nrm[:, :], op0=OP.mult, op1=OP.add)
                # ELU = max(x, exp(min(x,0)) - 1)
                neg = fin.tile([128, HF], fp32, tag="neg")
                nc.vector.tensor_scalar_min(out=neg[:, :], in0=nrm[:, :], scalar1=0.0)
                nc.scalar.activation(neg[:, :], neg[:, :], AF.Exp)
                res = fin.tile([128, HF], fp32, tag="res")
                nc.vector.scalar_tensor_tensor(
                    out=res[:, :], in0=neg[:, :], scalar=-1.0, in1=nrm[:, :],
                    op0=OP.add, op1=OP.max)
                nc.scalar.dma_start(out_d[base_row:base_row + nrows, :],
                                    res[:nrows, :])

            qn = 0
            for i in range(NBLK + 2):
                if i < NBLK:
                    qn = stage_fetch(i, qn)
                if 1 <= i <= NBLK:
                    stage_compute(i - 1)
                if i >= 2:
                    stage_tail(i - 2)

    nc.compile()

    in_maps = []
    for c in range(N_CORES):
        in_maps.append({
            "nft": _pad_cols(NFT, NPAD), "wt": WT.astype(bfnp),
            "m12": M12.astype(bfnp), "brep": b_rep, "iota": iota_rep,
            "iotap": iotaP_i8,
            "idx16": idx16_np[c], "rowid": rowid_bf[c], "rowt": rowT_np[c],
            "nft_own": nft_own_np[c], "deg_own": deg_own_np[c],
        })
    import os
    trace = bool(os.environ.get("GAT_TRACE"))
    if trace:
        _install_ntff_hook()
    res = run_bass_kernel_spmd(nc, in_maps, list(range(N_CORES)), trace=trace)
    global _last_results
    _last_results = res
    out = np.concatenate([res.results[c]["out"] for c in range(N_CORES)], axis=0)
    return out


def _pad_cols(arr, cols):
    if arr.shape[1] == cols:
        return arr
    out = np.zeros((arr.shape[0], cols), dtype=arr.dtype)
    out[:, :arr.shape[1]] = arr
    return out


# revision 22
# speedup vs baseline: 1.5789x; 1.0510x over previous
"""Multi-head GAT layer on 8 Trainium2 NeuronCores (Bass/Tile SPMD kernel).

Strategy (edge-parallel, target-sharded):
  - Edges sorted by target, sharded across 8 cores by contiguous target
    ranges (N/8 nodes each): softmax + aggregation are core-local.
  - Phase 1a (replicated on every core): one bf16 PE pass over the node
    features builds a per-node table row [ h (128) | s2 (8) | s1 (8) ]
    (bf16, 512B rows) where h = NF @ W.T + b and s1/s2 are the per-node
    attention scores h . a1 / h . a2 (fused into the same matmul via
    W.T @ A12).
  - Phase 1b (per-core data, same program): the core's own 6250 target
    rows are recomputed into a resident SBUF table (fp32, including
    degree) so phase 2 needs no self-row gather at all.
  - Phase 2, software-pipelined per 128-target block:
      A: edge slots (padded to 128-slot tiles, sorted by src) fetched
         with dma_gather (int16 indices + static per-group base, 4 SWDGE
         queues, 64KB descriptor rings);
      B: slot->target one-hots built on DVE: oh from the resident rowid
         table, ohT from a host-uploaded free-axis rowid pattern (int8)
         so no PE transposes are needed;
      C: s1-per-slot via small PE matmuls against ohT;
      D: scores z = s1e+s2, ex = exp(lrelu(z)) (DVE+ACT), ex expanded
         across F_OUT on ACT so the big DVE multiply runs contiguous;
         a single PE matmul per tile accumulates [Msg | ex] into PSUM;
      E: tail = softmax division, skip term from the SBUF own-table,
         ELU as max(x, exp(min(x,0))-1), contiguous output write.
    Stages are emitted skewed (A/B/C for block i, D for i-1, E for i-2)
    so each in-order engine queue interleaves independent blocks.
"""

import numpy as np

N_CORES = 8
_last_results = None  # BassKernelResults of the most recent run (for harnesses)


def _install_ntff_hook():
    """Register the axon NTFF profiling hook if the image lacks antenv.axon_hooks."""
    import sys, types
    try:
        from antenv.axon_hooks import get_axon_ntff_profile_hook  # noqa: F401
        return
    except ImportError:
        pass
    try:
        mod = types.ModuleType("antenv.axon_hooks")
        holder = [None]
        mod.set_axon_ntff_profile_hook = lambda h: holder.__setitem__(0, h)
        mod.get_axon_ntff_profile_hook = lambda: holder[0]
        sys.modules["antenv.axon_hooks"] = mod
        from trn_agent_boot.trn_boot import _ntff_profile_via_ctypes
        mod.set_axon_ntff_profile_hook(
            _ntff_profile_via_ctypes("/opt/axon/libaxon_pjrt.so"))
    except Exception:
        sys.modules.pop("antenv.axon_hooks", None)


def kernel(node_features, edge_index, W, b, a):
    return gat_multicore(
        np.asarray(node_features, dtype=np.float32),
        np.asarray(edge_index, dtype=np.int32),
        np.asarray(W, dtype=np.float32),
        np.asarray(b, dtype=np.float32),
        np.asarray(a, dtype=np.float32),
    )


def gat_multicore(nf, ei, W, b, a, slope=0.2):
    import sys
    if "/opt/trn_rl_repo" not in sys.path:
        sys.path.insert(0, "/opt/trn_rl_repo")
    import ml_dtypes
    import concourse.bacc as bacc
    import concourse.tile as tile
    import concourse.mybir as mybir
    from concourse import library_config
    from concourse.bass_utils import run_bass_kernel_spmd
    from contextlib import ExitStack

    fp32 = mybir.dt.float32
    bf16 = mybir.dt.bfloat16
    i16 = mybir.dt.int16
    AF = mybir.ActivationFunctionType
    OP = mybir.AluOpType
    bfnp = ml_dtypes.bfloat16

    N, F_IN = nf.shape
    E = ei.shape[1]
    HF = W.shape[0]               # H * F_OUT
    F_OUT = a.shape[0] // 2
    H = HF // F_OUT
    assert F_IN == 128 and HF == 128, "kernel assumes 128 in/out features"
    assert N % N_CORES == 0
    NPC = N // N_CORES            # targets per core
    NBLK = (NPC + 127) // 128     # 128-target blocks per core
    GRP = 8                       # max tiles per gather group
    ROW = 256                     # bf16 elements per table row (512 B)
    SPAN = 30000                  # max int16 index span per gather group

    # ---------------- host prep: weights ----------------
    WT = np.ascontiguousarray(W.T)                       # [F_IN, HF]
    # A12 column order: [s2 (a2) | s1 (a1)] to match the table row layout
    A12 = np.zeros((HF, 2 * H), dtype=np.float32)
    for hd in range(H):
        A12[hd * F_OUT:(hd + 1) * F_OUT, hd] = a[F_OUT:]        # s2
        A12[hd * F_OUT:(hd + 1) * F_OUT, H + hd] = a[:F_OUT]    # s1
    M12 = (WT @ A12).astype(np.float32)                  # [F_IN, 2H]
    b12 = (b @ A12).astype(np.float32)                   # [2H]
    b_ext = np.concatenate([b, b12]).astype(np.float32)  # [144]
    b_rep = np.broadcast_to(b_ext, (128, HF + 2 * H)).copy()
    NFT = np.ascontiguousarray(nf.T).astype(bfnp)        # [F_IN, N] bf16

    # ---------------- host prep: graph structure ----------------
    src, tgt = ei[0].astype(np.int64), ei[1].astype(np.int64)
    order = np.argsort(tgt, kind="stable")
    ssrc, stgt = src[order], tgt[order]
    deg_full = np.bincount(tgt, minlength=N).astype(np.float32)
    n_nt = (N + 127) // 128
    NPAD = n_nt * 128             # h_tab rows incl. zero padding

    blk_bounds = []
    for c in range(N_CORES):
        bounds = [c * NPC + bb * 128 for bb in range(NBLK)] + [(c + 1) * NPC]
        blk_bounds.append(np.searchsorted(stgt, bounds))
    cnt = np.array([[blk_bounds[c][bb + 1] - blk_bounds[c][bb]
                     for bb in range(NBLK)] for c in range(N_CORES)])
    # edge tiles per block (uniform across cores)
    n_tiles_blk = np.maximum(1, (cnt.max(axis=0) + 127) // 128)
    NT = int(n_tiles_blk.sum())
    t_ofs_blk = np.concatenate([[0], np.cumsum(n_tiles_blk)]).astype(int)

    # Per-core slot arrays; tile t slot p = slot index t*128+p of the block.
    srcs_all = np.zeros((N_CORES, 128, NT), dtype=np.int64)
    rowid_np = np.full((N_CORES, 128, NT), -1.0, dtype=np.float32)
    for c in range(N_CORES):
        for bb in range(NBLK):
            lo, hi = blk_bounds[c][bb], blk_bounds[c][bb + 1]
            nslot = hi - lo
            base_node = c * NPC + bb * 128
            t0 = int(t_ofs_blk[bb])
            net = int(n_tiles_blk[bb])
            ne = net * 128
            if nslot > 0:
                o2 = np.argsort(ssrc[lo:hi], kind="stable")
                s_blk = ssrc[lo:hi][o2]
                pad_val = int(s_blk[-1])
                fl_s = np.full(ne, pad_val, dtype=np.int64)
                fl_r = np.full(ne, -1.0, dtype=np.float32)
                fl_s[:nslot] = s_blk
                fl_r[:nslot] = (stgt[lo:hi][o2] - base_node).astype(np.float32)
                srcs_all[c, :, t0:t0 + net] = fl_s.reshape(net, 128).T
                rowid_np[c, :, t0:t0 + net] = fl_r.reshape(net, 128).T
            # else: pad filled below from other cores
    for bb in range(NBLK):
        t0 = int(t_ofs_blk[bb])
        net = int(n_tiles_blk[bb])
        nonempty = [c for c in range(N_CORES) if cnt[c][bb] > 0]
        if nonempty and len(nonempty) < N_CORES:
            ref = int(srcs_all[nonempty[0], 0, t0])
            for c in range(N_CORES):
                if cnt[c][bb] == 0:
                    srcs_all[c, :, t0:t0 + net] = ref

    # Gather groups: consecutive tiles of one block, <= GRP tiles,
    # cross-core index span <= SPAN.
    groups = []          # (block, tile_lo, n_tiles, base)
    for bb in range(NBLK):
        net = int(n_tiles_blk[bb])
        t0 = int(t_ofs_blk[bb])
        t = 0
        while t < net:
            best = 1
            for w in range(2, min(GRP, net - t) + 1):
                sl = srcs_all[:, :, t0 + t:t0 + t + w]
                if sl.max() - sl.min() > SPAN:
                    break
                best = w
            sl = srcs_all[:, :, t0 + t:t0 + t + best]
            assert sl.max() - sl.min() <= 32000, "single tile span too large"
            groups.append((bb, t, best, int(sl.min())))
            t += best
    groups_by_block = [[] for _ in range(NBLK)]
    for gi, g in enumerate(groups):
        groups_by_block[g[0]].append((gi,) + g[1:])

    g_cols = [(g[2] * 128) // 16 for g in groups]
    g_col_ofs = np.concatenate([[0], np.cumsum(g_cols)]).astype(int)
    IDXC = int(g_col_ofs[-1])
    idx16_np = np.zeros((N_CORES, 128, IDXC), dtype=np.int16)
    for c in range(N_CORES):
        for gi, (bb, tl, w, base) in enumerate(groups):
            t0 = int(t_ofs_blk[bb]) + tl
            rel = (srcs_all[c, :, t0:t0 + w] - base).astype(np.int16)  # [128, w]
            flat = rel.T.reshape(-1)                 # slot order t*128+p
            wrapped = flat.reshape(-1, 16).T         # [16, w*128/16]
            idx16_np[c, :, g_col_ofs[gi]:g_col_ofs[gi + 1]] = np.tile(wrapped, (8, 1))

    # Both one-hot orientations, host-built and uploaded as fp8 (0/1 exact):
    #   oh [slot_p, t*128+tgt]  — lhsT of the accumulation matmul
    #   ohT[tgt_p,  t*128+slot] — lhsT of the s1-per-slot matmul
    f8np = ml_dtypes.float8_e4m3fn
    oh_np = np.zeros((N_CORES, 128, NT * 128), dtype=f8np)
    ohT_np = np.zeros((N_CORES, 128, NT * 128), dtype=f8np)
    for c in range(N_CORES):
        rid = rowid_np[c].astype(np.int64)          # [slot, t], -1 = invalid
        for t in range(NT):
            valid = rid[:, t] >= 0
            slots = np.nonzero(valid)[0]
            tgts = rid[slots, t]
            oh_np[c, slots, t * 128 + tgts] = 1.0
            ohT_np[c, tgts, t * 128 + slots] = 1.0

    # Per-core own-node inputs for phase 1b (the core's target rows)
    OWNW = NBLK * 128
    nft_own_np = np.zeros((N_CORES, 128, OWNW), dtype=bfnp)
    deg_own_np = np.zeros((N_CORES, 128, NBLK), dtype=bfnp)
    for c in range(N_CORES):
        nft_own_np[c, :, :NPC] = NFT[:, c * NPC:(c + 1) * NPC]
        dcol = np.zeros(OWNW, dtype=np.float32)
        dcol[:NPC] = deg_full[c * NPC:(c + 1) * NPC]
        deg_own_np[c] = dcol.reshape(NBLK, 128).T.astype(bfnp)

    # ---------------- build the SPMD program ----------------
    # 48 KiB SWDGE scratch -> 3072-descriptor rings per queue so a 1024-idx
    # gather never fills a ring and the Q7 doesn't stall in await_space.
    nc = bacc.Bacc("TRN2", target_bir_lowering=False, debug=False,
                   num_devices=N_CORES, num_swdge_queues=4,
                   dynamic_dma_scratch_size=49152)

    f8 = mybir.dt.float8e4
    nft_d = nc.dram_tensor("nft", [128, NPAD], bf16, kind="ExternalInput").ap()
    wt_d = nc.dram_tensor("wt", [128, HF], bf16, kind="ExternalInput").ap()
    m12_d = nc.dram_tensor("m12", [128, 2 * H], bf16, kind="ExternalInput").ap()
    brep_d = nc.dram_tensor("brep", [128, HF + 2 * H], fp32, kind="ExternalInput").ap()
    idx16_d = nc.dram_tensor("idx16", [128, IDXC], i16, kind="ExternalInput").ap()
    oh_d = nc.dram_tensor("ohp", [128, NT * 128], f8, kind="ExternalInput").ap()
    oht_d = nc.dram_tensor("ohtp", [128, NT * 128], f8, kind="ExternalInput").ap()
    nfto_d = nc.dram_tensor("nft_own", [128, OWNW], bf16, kind="ExternalInput").ap()
    dego_d = nc.dram_tensor("deg_own", [128, NBLK], bf16, kind="ExternalInput").ap()

    h_tab = nc.dram_tensor("h_tab", [NPAD, ROW], bf16).ap()
    out_d = nc.dram_tensor("out", [NPC, HF], fp32, kind="ExternalOutput").ap()

    CW = HF + H       # 136: [Msg | ex] combo width
    SW = HF + 2 * H   # 144: phase-1 psum width
    OSW = SW + 1      # own-row width incl. deg
    MAXT = int(n_tiles_blk.max())

    with tile.TileContext(nc) as tc:
        with ExitStack() as ctx:
            cpool = ctx.enter_context(tc.tile_pool(name="consts", bufs=1))
            p1 = ctx.enter_context(tc.tile_pool(name="p1", bufs=3))
            p1ps = ctx.enter_context(tc.tile_pool(name="p1ps", bufs=2, space="PSUM"))
            gp = ctx.enter_context(tc.tile_pool(name="gather", bufs=3))
            mp = ctx.enter_context(tc.tile_pool(name="meta", bufs=3))
            ps_acc = ctx.enter_context(tc.tile_pool(name="ps_acc", bufs=2, space="PSUM"))
            ps_z = ctx.enter_context(tc.tile_pool(name="ps_z", bufs=2, space="PSUM"))
            fin = ctx.enter_context(tc.tile_pool(name="fin", bufs=4))

            nc.gpsimd.load_library(library_config.mlp)

            wt_sb = cpool.tile([128, HF], bf16)
            nc.sync.dma_start(wt_sb[:], wt_d[:])
            m12_sb = cpool.tile([128, 2 * H], bf16)
            nc.sync.dma_start(m12_sb[:], m12_d[:])
            brep_sb = cpool.tile([128, SW], fp32)
            nc.sync.dma_start(brep_sb[:], brep_d[:])
            idx_sb = cpool.tile([128, IDXC], i16)
            nc.sync.dma_start(idx_sb[:], idx16_d[:])
            dego_sb = cpool.tile([128, NBLK], bf16)
            nc.sync.dma_start(dego_sb[:], dego_d[:])
            # SBUF-resident own-row table [h | s2 | s1 | deg] fp32
            own_sb = cpool.tile([128, NBLK, OSW], fp32)

            b_is_zero = not np.any(b_ext)

            # ---------- phase 1b: own rows -> resident SBUF table ----------
            for ob0 in range(0, NBLK, 2):
                nk = min(2, NBLK - ob0)
                nfo = p1.tile([128, 256], bf16, tag="nfo")
                nc.sync.dma_start(nfo[:, :nk * 128],
                                  nfto_d[:, ob0 * 128:ob0 * 128 + nk * 128])
                ps = p1ps.tile([128, 2, SW], fp32, space="PSUM", tag="p1ps")
                for k in range(nk):
                    nc.tensor.matmul(ps[:, k, 0:HF],
                                     lhsT=nfo[:, k * 128:(k + 1) * 128],
                                     rhs=wt_sb[:], start=True, stop=True)
                    nc.tensor.matmul(ps[:, k, HF:SW],
                                     lhsT=nfo[:, k * 128:(k + 1) * 128],
                                     rhs=m12_sb[:], start=True, stop=True)
                if b_is_zero:
                    nc.vector.tensor_copy(own_sb[:, ob0:ob0 + nk, 0:SW],
                                          ps[:, :nk, :])
                else:
                    nc.vector.tensor_tensor(
                        out=own_sb[:, ob0:ob0 + nk, 0:SW], in0=ps[:, :nk, :],
                        in1=brep_sb[:].unsqueeze(1).broadcast_to([128, nk, SW]),
                        op=OP.add)
                nc.vector.tensor_copy(own_sb[:, ob0:ob0 + nk, SW:SW + 1],
                                      dego_sb[:, ob0:ob0 + nk].unsqueeze(2))

            # ---------- phase 1a: full h table (replicated) ----------
            # One 512-node chunk per iteration: 4 matmul pairs into two PSUM
            # tiles, PSUM->SBUF casts alternating DVE/ACT, one table write.
            CH = 512
            for j0 in range(0, NPAD, CH):
                w = min(CH, NPAD - j0)
                nfc = p1.tile([128, CH], bf16, tag="nfc")
                nc.sync.dma_start(nfc[:, :w], nft_d[:, j0:j0 + w])
                nkc = (w + 127) // 128
                hrow = p1.tile([128, 4, ROW], bf16, tag="hrow")
                for k0 in range(0, w, 256):
                    kw2 = min(256, w - k0)
                    nk = (kw2 + 127) // 128
                    ps = p1ps.tile([128, 2, SW], fp32, space="PSUM", tag="p1ps")
                    for k in range(nk):
                        kk = k0 + k * 128
                        nc.tensor.matmul(ps[:, k, 0:HF],
                                         lhsT=nfc[:, kk:kk + 128],
                                         rhs=wt_sb[:], start=True, stop=True)
                        nc.tensor.matmul(ps[:, k, HF:SW],
                                         lhsT=nfc[:, kk:kk + 128],
                                         rhs=m12_sb[:], start=True, stop=True)
                    ko = k0 // 128
                    if b_is_zero and k0 == 0:
                        nc.vector.tensor_copy(hrow[:, ko:ko + nk, 0:SW],
                                              ps[:, :nk, :])
                    elif b_is_zero:
                        nc.scalar.copy(hrow[:, ko:ko + nk, 0:SW], ps[:, :nk, :])
                    else:
                        nc.vector.tensor_tensor(
                            out=hrow[:, ko:ko + nk, 0:SW], in0=ps[:, :nk, :],
                            in1=brep_sb[:].unsqueeze(1).broadcast_to([128, nk, SW]),
                            op=OP.add)
                nc.scalar.dma_start(
                    h_tab[j0:j0 + w, :].rearrange("(k p) r -> p k r", k=nkc),
                    hrow[:, :nkc, :])

            # ---------- phase 2: edge processing (software-pipelined) ----------
            blk_state = {}

            def stage_fetch(bb, qn0):
                """A: dma_gather the block's edge rows; B: build both one-hot
                orientations; C: s1-per-slot matmuls."""
                net = int(n_tiles_blk[bb])
                t0 = int(t_ofs_blk[bb])
                qn = qn0

                G = gp.tile([128, MAXT, ROW], bf16, tag="G")
                for gi, tl, wdt, base in groups_by_block[bb]:
                    nc.gpsimd.dma_gather(
                        out_ap=G[:, tl:tl + wdt, :],
                        in_ap=h_tab[base:, :],
                        idxs_ap=idx_sb[:, g_col_ofs[gi]:g_col_ofs[gi + 1]],
                        num_idxs=wdt * 128, num_idxs_reg=wdt * 128,
                        elem_size=ROW, queue_num=qn % 4)
                    qn += 1

                # both one-hot orientations stream in as fp8 matmul weights
                oh = gp.tile([128, MAXT, 128], f8, tag="oh")
                nc.sync.dma_start(oh[:, :net, :],
                                  oh_d[:, t0 * 128:(t0 + net) * 128])
                ohT = gp.tile([128, MAXT, 128], f8, tag="ohT")
                nc.sync.dma_start(ohT[:, :net, :],
                                  oht_d[:, t0 * 128:(t0 + net) * 128])

                # s1 of the block's targets (bf16 rhs), then s1e per slot
                s1bf = mp.tile([128, H], bf16, tag="s1bf")
                nc.vector.tensor_copy(s1bf[:], own_sb[:, bb, SW - H:SW])
                zps = ps_z.tile([128, MAXT, H], fp32, space="PSUM", tag="zps")
                for t in range(net):
                    nc.tensor.matmul(zps[:, t, :], lhsT=ohT[:, t, :],
                                     rhs=s1bf[:], start=True, stop=True)

                blk_state[bb] = (G, oh, zps, qn0)
                return qn

            def stage_compute(bb):
                """D: scores, weights, weighted-message accumulation."""
                net = int(n_tiles_blk[bb])
                G, oh, zps, _ = blk_state[bb]

                acc = ps_acc.tile([128, CW], fp32, space="PSUM", tag="acc")
                me = gp.tile([128, MAXT, CW], bf16, tag="me")

                z_sb = mp.tile([128, MAXT, H], fp32, tag="z_sb")
                nc.vector.tensor_tensor(out=z_sb[:, :net, :], in0=zps[:, :net, :],
                                        in1=G[:, :net, HF:HF + H], op=OP.add)
                ext = mp.tile([128, MAXT, H], bf16, tag="ext")
                nc.vector.scalar_tensor_tensor(
                    out=ext[:, :net, :], in0=z_sb[:, :net, :], scalar=slope,
                    in1=z_sb[:, :net, :], op0=OP.mult, op1=OP.max)
                nc.scalar.activation(ext[:, :net, :], ext[:, :net, :], AF.Exp)
                nc.vector.tensor_copy(me[:, :net, HF:CW], ext[:, :net, :])
                # expand ex across F_OUT on ACT so the big DVE multiply is
                # contiguous x contiguous (2x bf16 rate)
                ex128 = gp.tile([128, MAXT, H, F_OUT], bf16, tag="ex128")
                nc.scalar.copy(
                    ex128[:, :net, :, :],
                    ext[:, :net, :].unsqueeze(3).broadcast_to(
                        [128, net, H, F_OUT]))
                nc.vector.tensor_tensor(
                    out=me[:, 0:net, 0:HF], in0=G[:, 0:net, 0:HF],
                    in1=ex128[:, 0:net, :, :], op=OP.mult)

                for t in range(net):
                    nc.tensor.matmul(acc[:, :], lhsT=oh[:, t, :],
                                     rhs=me[:, t, :],
                                     start=(t == 0), stop=(t == net - 1))
                blk_state[bb] = acc

            def stage_tail(bb):
                """E: softmax division, skip term, ELU, output write."""
                base_row = bb * 128
                nrows = min(128, NPC - base_row)
                acc = blk_state.pop(bb)

                rec = fin.tile([128, H], fp32, tag="rec")
                nc.vector.tensor_scalar_add(out=rec[:, :], in0=acc[:, HF:CW],
                                            scalar1=1e-30)
                nc.vector.reciprocal(rec[:, :], rec[:, :])
                nrm = fin.tile([128, HF], fp32, tag="nrm")
                nc.vector.tensor_tensor(
                    out=nrm[:, :], in0=acc[:, 0:HF],
                    in1=rec[:].unsqueeze(2).broadcast_to([128, H, F_OUT]),
                    op=OP.mult)
                # += deg * h_own (fp32, exact skip term)
                nc.vector.scalar_tensor_tensor(
                    out=nrm[:, :], in0=own_sb[:, bb, 0:HF],
                    scalar=own_sb[:, bb, SW:SW + 1],
                    in1=nrm[:, :], op0=OP.mult, op1=OP.add)
                # ELU = max(x, exp(min(x,0)) - 1)
                neg = fin.tile([128, HF], fp32, tag="neg")
                nc.vector.tensor_scalar_min(out=neg[:, :], in0=nrm[:, :], scalar1=0.0)
                nc.scalar.activation(neg[:, :], neg[:, :], AF.Exp)
                res = fin.tile([128, HF], fp32, tag="res")
                nc.vector.scalar_tensor_tensor(
                    out=res[:, :], in0=neg[:, :], scalar=-1.0, in1=nrm[:, :],
                    op0=OP.add, op1=OP.max)
                nc.scalar.dma_start(out_d[base_row:base_row + nrows, :],
                                    res[:nrows, :])

            qn = 0
            for i in range(NBLK + 2):
                if i < NBLK:
                    qn = stage_fetch(i, qn)
                if 1 <= i <= NBLK:
                    stage_compute(i - 1)
                if i >= 2:
                    stage_tail(i - 2)

    nc.compile()

    in_maps = []
    for c in range(N_CORES):
        in_maps.append({
            "nft": _pad_cols(NFT, NPAD), "wt": WT.astype(bfnp),
            "m12": M12.astype(bfnp), "brep": b_rep,
            "idx16": idx16_np[c], "ohp": oh_np[c], "ohtp": ohT_np[c],
            "nft_own": nft_own_np[c], "deg_own": deg_own_np[c],
        })
    import os
    trace = bool(os.environ.get("GAT_TRACE"))
    if trace:
        _install_ntff_hook()
    res = run_bass_kernel_spmd(nc, in_maps, list(range(N_CORES)), trace=trace)
    global _last_results
    _last_results = res
    out = np.concatenate([res.results[c]["out"] for c in range(N_CORES)], axis=0)
    return out


def _pad_cols(arr, cols):
    if arr.shape[1] == cols:
        return arr
    out = np.zeros((arr.shape[0], cols), dtype=arr.dtype)
    out[:, :arr.shape[1]] = arr
    return out


# revision 25
# speedup vs baseline: 1.5892x; 1.0065x over previous
"""Multi-head GAT layer on 8 Trainium2 NeuronCores (Bass/Tile SPMD kernel).

Strategy (edge-parallel, target-sharded):
  - Edges sorted by target, sharded across 8 cores by contiguous target
    ranges (N/8 nodes each): softmax + aggregation are core-local.
  - Phase 1a (replicated on every core): one bf16 PE pass over the node
    features builds a per-node table row [ h (128) | s2 (8) | s1 (8) ]
    (bf16, 512B rows) where h = NF @ W.T + b and s1/s2 are the per-node
    attention scores h . a1 / h . a2 (fused into the same matmul via
    W.T @ A12).
  - Phase 1b (per-core data, same program): the core's own 6250 target
    rows are recomputed into a resident SBUF table (fp32, including
    degree) so phase 2 needs no self-row gather at all.
  - Phase 2, software-pipelined per 128-target block:
      A: edge slots (padded to 128-slot tiles, sorted by src) fetched
         with dma_gather (int16 indices + static per-group base, 4 SWDGE
         queues, 64KB descriptor rings);
      B: slot->target one-hots built on DVE: oh from the resident rowid
         table, ohT from a host-uploaded free-axis rowid pattern (int8)
         so no PE transposes are needed;
      C: s1-per-slot via small PE matmuls against ohT;
      D: scores z = s1e+s2, ex = exp(lrelu(z)) (DVE+ACT), ex expanded
         across F_OUT on ACT so the big DVE multiply runs contiguous;
         a single PE matmul per tile accumulates [Msg | ex] into PSUM;
      E: tail = softmax division, skip term from the SBUF own-table,
         ELU as max(x, exp(min(x,0))-1), contiguous output write.
    Stages are emitted skewed (A/B/C for block i, D for i-1, E for i-2)
    so each in-order engine queue interleaves independent blocks.
"""

import numpy as np

N_CORES = 8
_last_results = None  # BassKernelResults of the most recent run (for harnesses)


def _install_ntff_hook():
    """Register the axon NTFF profiling hook if the image lacks antenv.axon_hooks."""
    import sys, types
    try:
        from antenv.axon_hooks import get_axon_ntff_profile_hook  # noqa: F401
        return
    except ImportError:
        pass
    try:
        mod = types.ModuleType("antenv.axon_hooks")
        holder = [None]
        mod.set_axon_ntff_profile_hook = lambda h: holder.__setitem__(0, h)
        mod.get_axon_ntff_profile_hook = lambda: holder[0]
        sys.modules["antenv.axon_hooks"] = mod
        from trn_agent_boot.trn_boot import _ntff_profile_via_ctypes
        mod.set_axon_ntff_profile_hook(
            _ntff_profile_via_ctypes("/opt/axon/libaxon_pjrt.so"))
    except Exception:
        sys.modules.pop("antenv.axon_hooks", None)


def kernel(node_features, edge_index, W, b, a):
    return gat_multicore(
        np.asarray(node_features, dtype=np.float32),
        np.asarray(edge_index, dtype=np.int32),
        np.asarray(W, dtype=np.float32),
        np.asarray(b, dtype=np.float32),
        np.asarray(a, dtype=np.float32),
    )


def gat_multicore(nf, ei, W, b, a, slope=0.2):
    import sys
    if "/opt/trn_rl_repo" not in sys.path:
        sys.path.insert(0, "/opt/trn_rl_repo")
    import ml_dtypes
    import concourse.bacc as bacc
    import concourse.tile as tile
    import concourse.mybir as mybir
    from concourse import library_config
    from concourse.bass_utils import run_bass_kernel_spmd
    from contextlib import ExitStack

    fp32 = mybir.dt.float32
    bf16 = mybir.dt.bfloat16
    i16 = mybir.dt.int16
    AF = mybir.ActivationFunctionType
    OP = mybir.AluOpType
    bfnp = ml_dtypes.bfloat16

    N, F_IN = nf.shape
    E = ei.shape[1]
    HF = W.shape[0]               # H * F_OUT
    F_OUT = a.shape[0] // 2
    H = HF // F_OUT
    assert F_IN == 128 and HF == 128, "kernel assumes 128 in/out features"
    assert N % N_CORES == 0
    NPC = N // N_CORES            # targets per core
    NBLK = (NPC + 127) // 128     # 128-target blocks per core
    GRP = 8                       # max tiles per gather group
    ROW = 256                     # bf16 elements per table row (512 B)
    SPAN = 30000                  # max int16 index span per gather group

    # ---------------- host prep: weights ----------------
    WT = np.ascontiguousarray(W.T)                       # [F_IN, HF]
    # A12 column order: [s2 (a2) | s1 (a1)] to match the table row layout
    A12 = np.zeros((HF, 2 * H), dtype=np.float32)
    for hd in range(H):
        A12[hd * F_OUT:(hd + 1) * F_OUT, hd] = a[F_OUT:]        # s2
        A12[hd * F_OUT:(hd + 1) * F_OUT, H + hd] = a[:F_OUT]    # s1
    M12 = (WT @ A12).astype(np.float32)                  # [F_IN, 2H]
    b12 = (b @ A12).astype(np.float32)                   # [2H]
    b_ext = np.concatenate([b, b12]).astype(np.float32)  # [144]
    b_rep = np.broadcast_to(b_ext, (128, HF + 2 * H)).copy()
    NFT = np.ascontiguousarray(nf.T).astype(bfnp)        # [F_IN, N] bf16

    # ---------------- host prep: graph structure ----------------
    src, tgt = ei[0].astype(np.int64), ei[1].astype(np.int64)
    order = np.argsort(tgt, kind="stable")
    ssrc, stgt = src[order], tgt[order]
    deg_full = np.bincount(tgt, minlength=N).astype(np.float32)
    n_nt = (N + 127) // 128
    NPAD = n_nt * 128             # h_tab rows incl. zero padding

    blk_bounds = []
    for c in range(N_CORES):
        bounds = [c * NPC + bb * 128 for bb in range(NBLK)] + [(c + 1) * NPC]
        blk_bounds.append(np.searchsorted(stgt, bounds))
    cnt = np.array([[blk_bounds[c][bb + 1] - blk_bounds[c][bb]
                     for bb in range(NBLK)] for c in range(N_CORES)])
    # edge tiles per block (uniform across cores)
    n_tiles_blk = np.maximum(1, (cnt.max(axis=0) + 127) // 128)
    NT = int(n_tiles_blk.sum())
    t_ofs_blk = np.concatenate([[0], np.cumsum(n_tiles_blk)]).astype(int)

    # Per-core slot arrays; tile t slot p = slot index t*128+p of the block.
    srcs_all = np.zeros((N_CORES, 128, NT), dtype=np.int64)
    rowid_np = np.full((N_CORES, 128, NT), -1.0, dtype=np.float32)
    for c in range(N_CORES):
        for bb in range(NBLK):
            lo, hi = blk_bounds[c][bb], blk_bounds[c][bb + 1]
            nslot = hi - lo
            base_node = c * NPC + bb * 128
            t0 = int(t_ofs_blk[bb])
            net = int(n_tiles_blk[bb])
            ne = net * 128
            if nslot > 0:
                o2 = np.argsort(ssrc[lo:hi], kind="stable")
                s_blk = ssrc[lo:hi][o2]
                pad_val = int(s_blk[-1])
                fl_s = np.full(ne, pad_val, dtype=np.int64)
                fl_r = np.full(ne, -1.0, dtype=np.float32)
                fl_s[:nslot] = s_blk
                fl_r[:nslot] = (stgt[lo:hi][o2] - base_node).astype(np.float32)
                srcs_all[c, :, t0:t0 + net] = fl_s.reshape(net, 128).T
                rowid_np[c, :, t0:t0 + net] = fl_r.reshape(net, 128).T
            # else: pad filled below from other cores
    for bb in range(NBLK):
        t0 = int(t_ofs_blk[bb])
        net = int(n_tiles_blk[bb])
        nonempty = [c for c in range(N_CORES) if cnt[c][bb] > 0]
        if nonempty and len(nonempty) < N_CORES:
            ref = int(srcs_all[nonempty[0], 0, t0])
            for c in range(N_CORES):
                if cnt[c][bb] == 0:
                    srcs_all[c, :, t0:t0 + net] = ref

    # Gather groups: consecutive tiles of one block, <= GRP tiles,
    # cross-core index span <= SPAN.
    groups = []          # (block, tile_lo, n_tiles, base)
    for bb in range(NBLK):
        net = int(n_tiles_blk[bb])
        t0 = int(t_ofs_blk[bb])
        t = 0
        while t < net:
            best = 1
            for w in range(2, min(GRP, net - t) + 1):
                sl = srcs_all[:, :, t0 + t:t0 + t + w]
                if sl.max() - sl.min() > SPAN:
                    break
                best = w
            sl = srcs_all[:, :, t0 + t:t0 + t + best]
            assert sl.max() - sl.min() <= 32000, "single tile span too large"
            groups.append((bb, t, best, int(sl.min())))
            t += best
    groups_by_block = [[] for _ in range(NBLK)]
    for gi, g in enumerate(groups):
        groups_by_block[g[0]].append((gi,) + g[1:])

    g_cols = [(g[2] * 128) // 16 for g in groups]
    g_col_ofs = np.concatenate([[0], np.cumsum(g_cols)]).astype(int)
    IDXC = int(g_col_ofs[-1])
    idx16_np = np.zeros((N_CORES, 128, IDXC), dtype=np.int16)
    for c in range(N_CORES):
        for gi, (bb, tl, w, base) in enumerate(groups):
            t0 = int(t_ofs_blk[bb]) + tl
            rel = (srcs_all[c, :, t0:t0 + w] - base).astype(np.int16)  # [128, w]
            flat = rel.T.reshape(-1)                 # slot order t*128+p
            wrapped = flat.reshape(-1, 16).T         # [16, w*128/16]
            idx16_np[c, :, g_col_ofs[gi]:g_col_ofs[gi + 1]] = np.tile(wrapped, (8, 1))

    # Both one-hot orientations, host-built and uploaded as fp8 (0/1 exact):
    #   oh [slot_p, t*128+tgt]  — lhsT of the accumulation matmul
    #   ohT[tgt_p,  t*128+slot] — lhsT of the s1-per-slot matmul
    f8np = ml_dtypes.float8_e4m3fn
    oh_np = np.zeros((N_CORES, 128, NT * 128), dtype=f8np)
    ohT_np = np.zeros((N_CORES, 128, NT * 128), dtype=f8np)
    for c in range(N_CORES):
        rid = rowid_np[c].astype(np.int64)          # [slot, t], -1 = invalid
        for t in range(NT):
            valid = rid[:, t] >= 0
            slots = np.nonzero(valid)[0]
            tgts = rid[slots, t]
            oh_np[c, slots, t * 128 + tgts] = 1.0
            ohT_np[c, tgts, t * 128 + slots] = 1.0

    # Per-core own-node inputs for phase 1b (the core's target rows)
    OWNW = NBLK * 128
    nft_own_np = np.zeros((N_CORES, 128, OWNW), dtype=bfnp)
    deg_own_np = np.zeros((N_CORES, 128, NBLK), dtype=bfnp)
    for c in range(N_CORES):
        nft_own_np[c, :, :NPC] = NFT[:, c * NPC:(c + 1) * NPC]
        dcol = np.zeros(OWNW, dtype=np.float32)
        dcol[:NPC] = deg_full[c * NPC:(c + 1) * NPC]
        deg_own_np[c] = dcol.reshape(NBLK, 128).T.astype(bfnp)

    # ---------------- build the SPMD program ----------------
    # 48 KiB SWDGE scratch -> 3072-descriptor rings per queue so a 1024-idx
    # gather never fills a ring and the Q7 doesn't stall in await_space.
    nc = bacc.Bacc("TRN2", target_bir_lowering=False, debug=False,
                   num_devices=N_CORES, num_swdge_queues=4,
                   dynamic_dma_scratch_size=32768)

    f8 = mybir.dt.float8e4
    nft_d = nc.dram_tensor("nft", [128, NPAD], bf16, kind="ExternalInput").ap()
    wt_d = nc.dram_tensor("wt", [128, HF], bf16, kind="ExternalInput").ap()
    m12_d = nc.dram_tensor("m12", [128, 2 * H], bf16, kind="ExternalInput").ap()
    brep_d = nc.dram_tensor("brep", [128, HF + 2 * H], fp32, kind="ExternalInput").ap()
    idx16_d = nc.dram_tensor("idx16", [128, IDXC], i16, kind="ExternalInput").ap()
    oh_d = nc.dram_tensor("ohp", [128, NT * 128], f8, kind="ExternalInput").ap()
    oht_d = nc.dram_tensor("ohtp", [128, NT * 128], f8, kind="ExternalInput").ap()
    nfto_d = nc.dram_tensor("nft_own", [128, OWNW], bf16, kind="ExternalInput").ap()
    dego_d = nc.dram_tensor("deg_own", [128, NBLK], bf16, kind="ExternalInput").ap()

    h_tab = nc.dram_tensor("h_tab", [NPAD, ROW], bf16).ap()
    out_d = nc.dram_tensor("out", [NPC, HF], fp32, kind="ExternalOutput").ap()

    CW = HF + H       # 136: [Msg | ex] combo width
    SW = HF + 2 * H   # 144: phase-1 psum width
    OSW = SW + 1      # own-row width incl. deg
    MAXT = int(n_tiles_blk.max())

    with tile.TileContext(nc) as tc:
        with ExitStack() as ctx:
            cpool = ctx.enter_context(tc.tile_pool(name="consts", bufs=1))
            p1 = ctx.enter_context(tc.tile_pool(name="p1", bufs=3))
            p1ps = ctx.enter_context(tc.tile_pool(name="p1ps", bufs=2, space="PSUM"))
            gpool = ctx.enter_context(tc.tile_pool(name="gbuf", bufs=5))
            gp = ctx.enter_context(tc.tile_pool(name="gather", bufs=3))
            mp = ctx.enter_context(tc.tile_pool(name="meta", bufs=4))
            ps_acc = ctx.enter_context(tc.tile_pool(name="ps_acc", bufs=2, space="PSUM"))
            ps_z = ctx.enter_context(tc.tile_pool(name="ps_z", bufs=3, space="PSUM"))
            fin = ctx.enter_context(tc.tile_pool(name="fin", bufs=4))

            nc.gpsimd.load_library(library_config.mlp)

            wt_sb = cpool.tile([128, HF], bf16)
            nc.sync.dma_start(wt_sb[:], wt_d[:])
            m12_sb = cpool.tile([128, 2 * H], bf16)
            nc.sync.dma_start(m12_sb[:], m12_d[:])
            brep_sb = cpool.tile([128, SW], fp32)
            nc.sync.dma_start(brep_sb[:], brep_d[:])
            idx_sb = cpool.tile([128, IDXC], i16)
            nc.sync.dma_start(idx_sb[:], idx16_d[:])
            dego_sb = cpool.tile([128, NBLK], bf16)
            nc.sync.dma_start(dego_sb[:], dego_d[:])
            # SBUF-resident own-row table [h | s2 | s1 | deg] fp32
            own_sb = cpool.tile([128, NBLK, OSW], fp32)

            b_is_zero = not np.any(b_ext)

            # ---------- phase 1b: own rows -> resident SBUF table ----------
            for ob0 in range(0, NBLK, 2):
                nk = min(2, NBLK - ob0)
                nfo = p1.tile([128, 256], bf16, tag="nfo")
                nc.sync.dma_start(nfo[:, :nk * 128],
                                  nfto_d[:, ob0 * 128:ob0 * 128 + nk * 128])
                ps = p1ps.tile([128, 2, SW], fp32, space="PSUM", tag="p1ps")
                for k in range(nk):
                    nc.tensor.matmul(ps[:, k, 0:HF],
                                     lhsT=nfo[:, k * 128:(k + 1) * 128],
                                     rhs=wt_sb[:], start=True, stop=True)
                    nc.tensor.matmul(ps[:, k, HF:SW],
                                     lhsT=nfo[:, k * 128:(k + 1) * 128],
                                     rhs=m12_sb[:], start=True, stop=True)
                if b_is_zero:
                    nc.vector.tensor_copy(own_sb[:, ob0:ob0 + nk, 0:SW],
                                          ps[:, :nk, :])
                else:
                    nc.vector.tensor_tensor(
                        out=own_sb[:, ob0:ob0 + nk, 0:SW], in0=ps[:, :nk, :],
                        in1=brep_sb[:].unsqueeze(1).broadcast_to([128, nk, SW]),
                        op=OP.add)
                nc.vector.tensor_copy(own_sb[:, ob0:ob0 + nk, SW:SW + 1],
                                      dego_sb[:, ob0:ob0 + nk].unsqueeze(2))

            # ---------- phase 1a: full h table (replicated) ----------
            # One 512-node chunk per iteration: 4 matmul pairs into two PSUM
            # tiles, PSUM->SBUF casts alternating DVE/ACT, one table write.
            CH = 512
            for j0 in range(0, NPAD, CH):
                w = min(CH, NPAD - j0)
                nfc = p1.tile([128, CH], bf16, tag="nfc")
                nc.sync.dma_start(nfc[:, :w], nft_d[:, j0:j0 + w])
                nkc = (w + 127) // 128
                hrow = p1.tile([128, 4, ROW], bf16, tag="hrow")
                for k0 in range(0, w, 256):
                    kw2 = min(256, w - k0)
                    nk = (kw2 + 127) // 128
                    ps = p1ps.tile([128, 2, SW], fp32, space="PSUM", tag="p1ps")
                    for k in range(nk):
                        kk = k0 + k * 128
                        nc.tensor.matmul(ps[:, k, 0:HF],
                                         lhsT=nfc[:, kk:kk + 128],
                                         rhs=wt_sb[:], start=True, stop=True)
                        nc.tensor.matmul(ps[:, k, HF:SW],
                                         lhsT=nfc[:, kk:kk + 128],
                                         rhs=m12_sb[:], start=True, stop=True)
                    ko = k0 // 128
                    if b_is_zero and k0 == 0:
                        nc.vector.tensor_copy(hrow[:, ko:ko + nk, 0:SW],
                                              ps[:, :nk, :])
                    elif b_is_zero:
                        nc.scalar.copy(hrow[:, ko:ko + nk, 0:SW], ps[:, :nk, :])
                    else:
                        nc.vector.tensor_tensor(
                            out=hrow[:, ko:ko + nk, 0:SW], in0=ps[:, :nk, :],
                            in1=brep_sb[:].unsqueeze(1).broadcast_to([128, nk, SW]),
                            op=OP.add)
                nc.scalar.dma_start(
                    h_tab[j0:j0 + w, :].rearrange("(k p) r -> p k r", k=nkc),
                    hrow[:, :nkc, :])

            # ---------- phase 2: edge processing (software-pipelined) ----------
            blk_state = {}

            def stage_fetch(bb, qn0):
                """A: dma_gather the block's edge rows; B: build both one-hot
                orientations; C: s1-per-slot matmuls."""
                net = int(n_tiles_blk[bb])
                t0 = int(t_ofs_blk[bb])
                qn = qn0

                G = gpool.tile([128, MAXT, ROW], bf16, tag="G")
                for gi, tl, wdt, base in groups_by_block[bb]:
                    nc.gpsimd.dma_gather(
                        out_ap=G[:, tl:tl + wdt, :],
                        in_ap=h_tab[base:, :],
                        idxs_ap=idx_sb[:, g_col_ofs[gi]:g_col_ofs[gi + 1]],
                        num_idxs=wdt * 128, num_idxs_reg=wdt * 128,
                        elem_size=ROW, queue_num=qn % 4)
                    qn += 1

                # both one-hot orientations stream in as fp8 matmul weights
                oh = gp.tile([128, MAXT, 128], f8, tag="oh")
                nc.sync.dma_start(oh[:, :net, :],
                                  oh_d[:, t0 * 128:(t0 + net) * 128])
                ohT = gp.tile([128, MAXT, 128], f8, tag="ohT")
                nc.sync.dma_start(ohT[:, :net, :],
                                  oht_d[:, t0 * 128:(t0 + net) * 128])

                # s1 of the block's targets (bf16 rhs), then s1e per slot
                s1bf = mp.tile([128, H], bf16, tag="s1bf")
                nc.vector.tensor_copy(s1bf[:], own_sb[:, bb, SW - H:SW])
                zps = ps_z.tile([128, MAXT, H], fp32, space="PSUM", tag="zps")
                for t in range(net):
                    nc.tensor.matmul(zps[:, t, :], lhsT=ohT[:, t, :],
                                     rhs=s1bf[:], start=True, stop=True)

                blk_state[bb] = (G, oh, zps, qn0)
                return qn

            def stage_compute(bb):
                """D: scores, weights, weighted-message accumulation."""
                net = int(n_tiles_blk[bb])
                G, oh, zps, _ = blk_state[bb]

                acc = ps_acc.tile([128, CW], fp32, space="PSUM", tag="acc")
                me = gp.tile([128, MAXT, CW], bf16, tag="me")

                z_sb = mp.tile([128, MAXT, H], fp32, tag="z_sb")
                nc.vector.tensor_tensor(out=z_sb[:, :net, :], in0=zps[:, :net, :],
                                        in1=G[:, :net, HF:HF + H], op=OP.add)
                ext = mp.tile([128, MAXT, H], bf16, tag="ext")
                nc.vector.scalar_tensor_tensor(
                    out=ext[:, :net, :], in0=z_sb[:, :net, :], scalar=slope,
                    in1=z_sb[:, :net, :], op0=OP.mult, op1=OP.max)
                nc.scalar.activation(ext[:, :net, :], ext[:, :net, :], AF.Exp)
                nc.vector.tensor_copy(me[:, :net, HF:CW], ext[:, :net, :])
                # expand ex across F_OUT on ACT so the big DVE multiply is
                # contiguous x contiguous (2x bf16 rate)
                ex128 = gp.tile([128, MAXT, H, F_OUT], bf16, tag="ex128")
                nc.scalar.copy(
                    ex128[:, :net, :, :],
                    ext[:, :net, :].unsqueeze(3).broadcast_to(
                        [128, net, H, F_OUT]))
                nc.vector.tensor_tensor(
                    out=me[:, 0:net, 0:HF], in0=G[:, 0:net, 0:HF],
                    in1=ex128[:, 0:net, :, :], op=OP.mult)

                for t in range(net):
                    nc.tensor.matmul(acc[:, :], lhsT=oh[:, t, :],
                                     rhs=me[:, t, :],
                                     start=(t == 0), stop=(t == net - 1))
                blk_state[bb] = acc

            def stage_tail(bb):
                """E: softmax division, skip term, ELU, output write."""
                base_row = bb * 128
                nrows = min(128, NPC - base_row)
                acc = blk_state.pop(bb)

                rec = fin.tile([128, H], fp32, tag="rec")
                nc.vector.tensor_scalar_add(out=rec[:, :], in0=acc[:, HF:CW],
                                            scalar1=1e-30)
                nc.vector.reciprocal(rec[:, :], rec[:, :])
                nrm = fin.tile([128, HF], fp32, tag="nrm")
                nc.vector.tensor_tensor(
                    out=nrm[:, :], in0=acc[:, 0:HF],
                    in1=rec[:].unsqueeze(2).broadcast_to([128, H, F_OUT]),
                    op=OP.mult)
                # += deg * h_own (fp32, exact skip term)
                nc.vector.scalar_tensor_tensor(
                    out=nrm[:, :], in0=own_sb[:, bb, 0:HF],
                    scalar=own_sb[:, bb, SW:SW + 1],
                    in1=nrm[:, :], op0=OP.mult, op1=OP.add)
                # ELU = max(x, exp(min(x,0)) - 1)
                neg = fin.tile([128, HF], fp32, tag="neg")
                nc.vector.tensor_scalar_min(out=neg[:, :], in0=nrm[:, :], scalar1=0.0)
                nc.scalar.activation(neg[:, :], neg[:, :], AF.Exp)
                res = fin.tile([128, HF], fp32, tag="res")
                nc.vector.scalar_tensor_tensor(
                    out=res[:, :], in0=neg[:, :], scalar=-1.0, in1=nrm[:, :],
                    op0=OP.add, op1=OP.max)
                nc.scalar.dma_start(out_d[base_row:base_row + nrows, :],
                                    res[:nrows, :])

            qn = 0
            for i in range(NBLK + 2):
                if i < NBLK:
                    qn = stage_fetch(i, qn)
                if 1 <= i <= NBLK:
                    stage_compute(i - 1)
                if i >= 2:
                    stage_tail(i - 2)

    nc.compile()

    in_maps = []
    for c in range(N_CORES):
        in_maps.append({
            "nft": _pad_cols(NFT, NPAD), "wt": WT.astype(bfnp),
            "m12": M12.astype(bfnp), "brep": b_rep,
            "idx16": idx16_np[c], "ohp": oh_np[c], "ohtp": ohT_np[c],
            "nft_own": nft_own_np[c], "deg_own": deg_own_np[c],
        })
    import os
    trace = bool(os.environ.get("GAT_TRACE"))
    if trace:
        _install_ntff_hook()
    res = run_bass_kernel_spmd(nc, in_maps, list(range(N_CORES)), trace=trace)
    global _last_results
    _last_results = res
    out = np.concatenate([res.results[c]["out"] for c in range(N_CORES)], axis=0)
    return out


def _pad_cols(arr, cols):
    if arr.shape[1] == cols:
        return arr
    out = np.zeros((arr.shape[0], cols), dtype=arr.dtype)
    out[:, :arr.shape[1]] = arr
    return out
